# revision 15
# baseline (speedup 1.0000x reference)
"""Trainium2 Bass kernel for nn_LocationEmbedding (GCN scatter-add + trajectory gather).

Strategy (8 NeuronCores, SPMD, two launches):
  Launch A (per core, owns nodes [k*12500, (k+1)*12500)):
    - deg via segmented reduce of host-padded edge weights, dinv = rsqrt(deg+1)
    - x = node_feat @ W per 128-row block (bf16 matmuls)
    Host assembles x_full [100000,128] bf16 and dinv_full [100000] f32.
  Launch B (per core, target-sharded edges incl. self-loops as real edges):
    - edges grouped by (target block, source bank = row % 4), padded to
      128-slot chunks; bulk dma_gather (int16 idxs, strided bank views of
      x_full) fetches all source rows in few large calls
    - per chunk: one fused DVE op builds the weighted one-hot
      (iota == cl) * w', w' = w * dinv[row] (computed on device), then a
      PSUM-accumulated 128x128x128 matmul scatters into the target block
    - block tail: road = relu(dinv_t * zp) via one fused mult+max DVE op
    - one big road write, one dma_gather for the trajectory rows
All arithmetic on device; host does sharding, padding, and index layout.
"""

import numpy as np
import ml_dtypes

import concourse.bass as bass
import concourse.bacc as bacc
import concourse.tile as tile
from concourse import mybir, library_config
from concourse.bass_utils import run_bass_kernel_spmd

BF16 = ml_dtypes.bfloat16
P = 128
N, E, D = 100000, 1600000, 128
NCORES = 8
NS = N // NCORES          # 12500 nodes per core
NB = (NS + P - 1) // P    # 98 target blocks per core
NSPAD = NB * P            # 12544
NBANK = 4                 # source banks (row % 4) so gather idxs fit int16
SBB = 7                   # target blocks per gather superblock
NSB = (NB + SBB - 1) // SBB

F32 = mybir.dt.float32
BF = mybir.dt.bfloat16
I16 = mybir.dt.int16

LAST_EXEC_NS = None
LAST_EXEC_PARTS = None
LAST_NCS = None  # (nca, ncb) for offline simulation


def _build_kernel_a(padw):
    nc = bacc.Bacc("TRN2", target_bir_lowering=False, debug=False)
    nfsT = nc.dram_tensor("nfsT", [P, NSPAD], BF, kind="ExternalInput")
    wsb = nc.dram_tensor("wsb", [P, P], BF, kind="ExternalInput")
    wpad = nc.dram_tensor("wpad", [P, NB * padw], F32, kind="ExternalInput")
    x_sh = nc.dram_tensor("x_sh", [P, NSPAD], BF, kind="ExternalOutput")
    dinv_sh = nc.dram_tensor("dinv_sh", [P, NB], F32, kind="ExternalOutput")
    with tile.TileContext(nc) as tc:
        with tc.tile_pool(name="sb", bufs=1) as sb, \
             tc.tile_pool(name="ps", bufs=4, space="PSUM") as ps:
            nf_sb = sb.tile([P, NSPAD], BF)
            nsplit = 7
            step = (NB // nsplit) * P
            for s in range(nsplit):
                lo, hi = s * step, min((s + 1) * step, NSPAD)
                nc.sync.dma_start(nf_sb[:, lo:hi], nfsT[:, lo:hi])
            w_sb = sb.tile([P, P], BF)
            nc.sync.dma_start(w_sb[:], wsb[:])
            wp_sb = sb.tile([P, NB * padw], F32)
            nc.sync.dma_start(wp_sb[:], wpad[:])

            deg = sb.tile([P, NB], F32)
            nc.vector.tensor_reduce(
                out=deg[:],
                in_=wp_sb[:].rearrange("p (b s) -> p b s", s=padw),
                axis=mybir.AxisListType.X,
                op=mybir.AluOpType.add,
            )
            nc.vector.tensor_scalar_add(deg[:], deg[:], 1.0)
            rec = sb.tile([P, NB], F32)
            nc.vector.reciprocal(rec[:], deg[:])
            dinv = sb.tile([P, NB], F32)
            nc.scalar.activation(dinv[:], rec[:], mybir.ActivationFunctionType.Sqrt)
            nc.sync.dma_start(dinv_sh[:], dinv[:])

            xout = sb.tile([P, NSPAD], BF)
            wchunk = 14  # blocks per x_sh writeback
            for b in range(NB):
                xp = ps.tile([P, P], F32, tag="xp")
                nc.tensor.matmul(xp[:], lhsT=nf_sb[:, b * P:(b + 1) * P],
                                 rhs=w_sb[:], start=True, stop=True)
                nc.vector.tensor_copy(xout[:, b * P:(b + 1) * P], xp[:])
                if (b + 1) % wchunk == 0 or b == NB - 1:
                    lo = (b // wchunk) * wchunk * P
                    nc.sync.dma_start(x_sh[:, lo:(b + 1) * P],
                                      xout[:, lo:(b + 1) * P])
    nc.compile()
    return nc


def _build_kernel_b(cbq, j2):
    """cbq: [NB][NBANK] chunk counts (identical across cores); j2: output
    gather chunks."""
    cbq = np.asarray(cbq)
    J = int(cbq.sum())
    TOT = J * P

    # enumeration order of chunk groups: (sb asc, q asc, b within sb asc)
    group_order = []
    call_specs = []  # (sb, q, nchunks, chunkbase)
    base = 0
    for sbi in range(NSB):
        blks = range(sbi * SBB, min((sbi + 1) * SBB, NB))
        for q in range(NBANK):
            nch = int(sum(cbq[b][q] for b in blks))
            if nch == 0:
                continue
            call_specs.append((sbi, q, nch, base))
            for b in blks:
                group_order.append((b, q, base))
                base += int(cbq[b][q])
    assert base == J
    # per (b,q): global chunk base
    gbase = np.full((NB, NBANK), -1, np.int64)
    for b, q, g0 in group_order:
        gbase[b][q] = g0

    nc = bacc.Bacc("TRN2", target_bir_lowering=False, debug=False)
    xfull = nc.dram_tensor("xfull", [N, P], BF, kind="ExternalInput")
    idxs = nc.dram_tensor("idxs", [P, TOT // 16], I16, kind="ExternalInput")
    oidx = nc.dram_tensor("oidx", [P, j2 * P // 16], I16, kind="ExternalInput")
    clp = nc.dram_tensor("clp", [P, J], F32, kind="ExternalInput")
    wdp = nc.dram_tensor("wdp", [P, J], F32, kind="ExternalInput")
    dvp = nc.dram_tensor("dvp", [P, J], F32, kind="ExternalInput")
    dinv_t = nc.dram_tensor("dinv_t", [P, NB], F32, kind="ExternalInput")
    out_packed = nc.dram_tensor("out_packed", [P, j2 * P], BF,
                                kind="ExternalOutput")

    # bank view of xfull: rows r with r % NBANK == q, idx = r // NBANK
    xview = xfull[:].rearrange("(n f) d -> n f d", f=NBANK)

    with tile.TileContext(nc) as tc:
        with tc.tile_pool(name="sb", bufs=1) as sb, \
             tc.tile_pool(name="gp", bufs=2) as gp, \
             tc.tile_pool(name="op", bufs=12) as op_, \
             tc.tile_pool(name="ps", bufs=4, space="PSUM") as ps:
            nc.gpsimd.load_library(library_config.mlp)

            # idx loads split per superblock so the first gather starts early
            idx_sb = sb.tile([P, TOT // 16], I16)
            sb_col = []  # (col0, col1) per superblock
            c0 = 0
            for sbi in range(NSB):
                ncols = 8 * sum(nch for s, q, nch, _ in call_specs if s == sbi)
                sb_col.append((c0, c0 + ncols))
                if ncols:
                    nc.sync.dma_start(idx_sb[:, c0:c0 + ncols],
                                      idxs[:, c0:c0 + ncols])
                c0 += ncols
            assert c0 == TOT // 16
            oix_sb = sb.tile([P, j2 * P // 16], I16)
            nc.sync.dma_start(oix_sb[:], oidx[:])
            cl_sb = sb.tile([P, J], F32)
            nc.sync.dma_start(cl_sb[:], clp[:])
            wd_sb = sb.tile([P, J], F32)
            nc.sync.dma_start(wd_sb[:], wdp[:])
            dv_sb = sb.tile([P, J], F32)
            nc.sync.dma_start(dv_sb[:], dvp[:])
            dt_sb = sb.tile([P, NB], F32)
            nc.sync.dma_start(dt_sb[:], dinv_t[:])

            # w' = w * dinv[row], one big fused op
            wf_sb = sb.tile([P, J], F32)
            nc.vector.tensor_tensor(out=wf_sb[:], in0=wd_sb[:], in1=dv_sb[:],
                                    op=mybir.AluOpType.mult)

            iota_i = sb.tile([P, P], mybir.dt.int32)
            nc.gpsimd.iota(iota_i[:], pattern=[[1, P]], channel_multiplier=0)
            iota_f = sb.tile([P, P], F32)
            nc.vector.tensor_copy(iota_f[:], iota_i[:])
            iota_bf = sb.tile([P, P], BF)
            nc.vector.tensor_copy(iota_bf[:], iota_f[:])

            road_sb = sb.tile([P, NSPAD], BF)

            ci = 0  # call index
            for sbi in range(NSB):
                blks = range(sbi * SBB, min((sbi + 1) * SBB, NB))
                gts = {}
                for q in range(NBANK):
                    if ci < len(call_specs) and call_specs[ci][0] == sbi \
                            and call_specs[ci][1] == q:
                        _, _, nch, cb0 = call_specs[ci]
                        ci += 1
                        gt = gp.tile([P, nch * P], BF, tag=f"g{q}")
                        nc.gpsimd.dma_gather(
                            gt[:].rearrange("p (j d) -> p j d", d=P),
                            xview[:, q, :],
                            idx_sb[:, cb0 * 8:(cb0 + nch) * 8],
                            nch * P, nch * P, P, elem_step=NBANK * P,
                            single_packet=False)
                        gts[q] = (gt, cb0)
                for b in blks:
                    tot = int(cbq[b].sum())
                    zp = ps.tile([P, P], F32, tag="zp")
                    ji = 0
                    for q in range(NBANK):
                        nch = int(cbq[b][q])
                        if nch == 0:
                            continue
                        gt, cb0 = gts[q]
                        g0 = int(gbase[b][q])
                        for i in range(nch):
                            gj = g0 + i
                            jl = gj - cb0
                            ohw = op_.tile([P, P], BF, tag="oh")
                            nc.vector.tensor_scalar(
                                ohw[:], iota_bf[:], cl_sb[:, gj:gj + 1],
                                wf_sb[:, gj:gj + 1],
                                mybir.AluOpType.is_equal, mybir.AluOpType.mult)
                            nc.tensor.matmul(
                                zp[:], lhsT=ohw[:],
                                rhs=gt[:, jl * P:(jl + 1) * P],
                                start=(ji == 0), stop=(ji == tot - 1))
                            ji += 1
                    nc.vector.tensor_scalar(
                        road_sb[:, b * P:(b + 1) * P], zp[:],
                        dt_sb[:, b:b + 1], 0.0,
                        mybir.AluOpType.mult, mybir.AluOpType.max)

            # trajectory gather straight from SBUF road (transposed output):
            # idx v -> partition v % 128, rank (block) v // 128
            og = sb.tile([P, j2 * P], BF)
            nc.gpsimd.dma_gather(
                og[:].rearrange("p (c n) -> p c n", c=1),
                road_sb[:], oix_sb[:], j2 * P, j2 * P, P,
                transpose=True, single_packet=False,
                sbuf_tokens_per_rank=P,
                sbuf_free_dim_per_rank=P * 2,
                sbuf_byte_offset=0)
            nc.sync.dma_start(out_packed[:], og[:])
    nc.compile()
    return nc


def kernel(**inputs):
    traj = np.asarray(inputs["traj_seqs"])[..., 0].astype(np.int64)
    seq_len = np.asarray(inputs["seq_len"]).astype(np.int64)
    nf = np.asarray(inputs["node_feat"], dtype=np.float32)
    ei = np.asarray(inputs["edge_index"]).astype(np.int64)
    ef = np.asarray(inputs["edge_feat"], dtype=np.float32)
    W = np.asarray(inputs["W"], dtype=np.float32)
    b = np.asarray(inputs["b"], dtype=np.float32)
    assert np.all(b == 0.0), "nonzero bias not wired into device path"

    row, col = ei[0], ei[1]
    owner = col // NS

    # ---------- per-core edge sets (self-loops as real edges) ----------
    loops_c = np.arange(NS, dtype=np.int64)
    core_edges = []
    cnts = np.zeros((NCORES, NB, NBANK), np.int64)
    for k in range(NCORES):
        m = owner == k
        ck = np.concatenate([col[m] - k * NS, loops_c])
        rk = np.concatenate([row[m], k * NS + loops_c])
        wk = np.concatenate([ef[m], np.ones(NS, np.float32)])
        bq = (ck // P) * NBANK + (rk % NBANK)
        srt = np.argsort(bq, kind="stable")
        ck, rk, wk, bq = ck[srt], rk[srt], wk[srt], bq[srt]
        np.add.at(cnts, (k, bq // NBANK, bq % NBANK), 1)
        core_edges.append((ck, rk, wk, bq))

    cbq = np.ceil(cnts.max(axis=0) / P).astype(np.int64)  # [NB, NBANK]
    J = int(cbq.sum())
    TOT = J * P

    # group enumeration must match _build_kernel_b
    gbase = np.zeros((NB, NBANK), np.int64)
    base = 0
    for sbi in range(NSB):
        blks = range(sbi * SBB, min((sbi + 1) * SBB, NB))
        for q in range(NBANK):
            nch = int(sum(cbq[bb][q] for bb in blks))
            if nch == 0:
                continue
            cb0 = base
            for bb in blks:
                gbase[bb][q] = base
                base += int(cbq[bb][q])
            # call base for (sb,q) is cb0; chunks of (b,q) start at gbase[b][q]
    assert base == J

    # trajectory selection
    flat = traj.reshape(-1)
    L = traj.shape[1]
    posmask = (np.arange(L)[None, :] < seq_len[:, None]).reshape(-1)
    oo = flat // NS
    sels = [np.where((oo == k) & posmask)[0] for k in range(NCORES)]
    j2 = max(1, int(np.ceil(max(len(s) for s in sels) / P)))

    # ---------- launch A ----------
    padw = 1
    for k in range(NCORES):
        m = owner == k
        c_loc = col[m] - k * NS
        padw = max(padw, int(np.bincount(c_loc, minlength=NS).max()))

    nf_bf = nf.astype(BF16)
    W_bf = np.ascontiguousarray(W.astype(BF16))
    in_maps_a = []
    for k in range(NCORES):
        m = owner == k
        c_loc = col[m] - k * NS
        w_loc = ef[m]
        cnt = np.bincount(c_loc, minlength=NS)
        starts = np.zeros(NS, np.int64)
        np.cumsum(cnt[:-1], out=starts[1:])
        srt = np.argsort(c_loc, kind="stable")
        cs, ws = c_loc[srt], w_loc[srt]
        posin = np.arange(len(cs)) - starts[cs]
        arr = np.zeros((NSPAD, padw), np.float32)
        arr[cs, posin] = ws
        wpad = np.ascontiguousarray(
            arr.reshape(NB, P, padw).transpose(1, 0, 2).reshape(P, NB * padw))
        nfsT = np.zeros((P, NSPAD), BF16)
        nfsT[:, :NS] = nf_bf[k * NS:(k + 1) * NS].T
        in_maps_a.append({"nfsT": nfsT, "wsb": W_bf, "wpad": wpad})

    nca = _build_kernel_a(padw)
    ra = run_bass_kernel_spmd(nca, in_maps_a, core_ids=list(range(NCORES)))

    # host reassembly: x_full rows, dinv_full
    x_full = np.zeros((N, P), BF16)
    dinv_full = np.zeros(N, np.float32)
    for k in range(NCORES):
        xs = ra.results[k]["x_sh"]          # [128, NSPAD]
        xr = xs.reshape(P, NB, P).transpose(1, 0, 2).reshape(NSPAD, P)
        x_full[k * NS:(k + 1) * NS] = xr[:NS]
        ds = ra.results[k]["dinv_sh"]       # [128, NB]
        dr = ds.T.reshape(NSPAD)
        dinv_full[k * NS:(k + 1) * NS] = dr[:NS]
    x_full = np.ascontiguousarray(x_full)

    # ---------- launch B ----------
    in_maps_b = []
    for k in range(NCORES):
        ck, rk, wk, bq = core_edges[k]
        bqcnt = np.bincount(bq, minlength=NB * NBANK).reshape(NB, NBANK)
        gstart = np.zeros(NB * NBANK, np.int64)
        np.cumsum(bqcnt.reshape(-1)[:-1], out=gstart[1:])
        pos = np.arange(len(ck)) - gstart[bq]
        gj = gbase[bq // NBANK, bq % NBANK] + pos // P
        par = pos % P

        clp = np.zeros((P, J), np.float32)
        wdp = np.zeros((P, J), np.float32)
        dvp = np.zeros((P, J), np.float32)
        clp[par, gj] = (ck % P).astype(np.float32)
        wdp[par, gj] = wk
        dvp[par, gj] = dinv_full[rk]

        idx_arr = np.zeros((16, TOT // 16), np.int16)
        f = gj * P + par                      # global flat slot
        idx_arr[f % 16, f // 16] = (rk // NBANK).astype(np.int16)
        idx_t = np.tile(idx_arr, (8, 1))      # replicate to 128 partitions

        lv = (flat[sels[k]] - k * NS).astype(np.int16)
        oarr = np.zeros((16, j2 * P // 16), np.int16)
        fo = np.arange(len(lv))
        oarr[fo % 16, fo // 16] = lv
        oidx_t = np.tile(oarr, (8, 1))

        in_maps_b.append({
            "xfull": x_full, "idxs": idx_t, "oidx": oidx_t,
            "clp": clp, "wdp": wdp, "dvp": dvp,
            "dinv_t": ra.results[k]["dinv_sh"],
        })

    ncb = _build_kernel_b(cbq, j2)
    rb = run_bass_kernel_spmd(ncb, in_maps_b, core_ids=list(range(NCORES)))

    global LAST_EXEC_NS, LAST_EXEC_PARTS, LAST_NCS
    LAST_NCS = (nca, ncb)
    LAST_EXEC_PARTS = (ra.exec_time_ns, rb.exec_time_ns)
    if ra.exec_time_ns and rb.exec_time_ns:
        LAST_EXEC_NS = ra.exec_time_ns + rb.exec_time_ns

    out = np.zeros((flat.shape[0], D), np.float32)
    for k in range(NCORES):
        if len(sels[k]):
            out[sels[k]] = rb.results[k]["out_packed"][:, :len(sels[k])].T.astype(
                np.float32)
    return out.reshape(traj.shape[0], L, D)


# revision 21
# speedup vs baseline: 1.0811x; 1.0811x over previous
"""Trainium2 Bass kernel for nn_LocationEmbedding (GCN scatter-add + trajectory gather).

Strategy (8 NeuronCores, SPMD, two launches):
  Launch A (per core, owns nodes [k*12500, (k+1)*12500)):
    - deg via segmented reduce of host-padded edge weights, dinv = rsqrt(deg+1)
    - x = node_feat @ W per 128-row block (bf16 matmuls)
    Host assembles x_full [100000,128] bf16 and dinv_full [100000] f32.
  Launch B (per core, target-sharded edges incl. self-loops as real edges):
    - edges grouped by (target block, source bank = row % 4), padded to
      128-slot chunks; bulk dma_gather (int16 idxs, strided bank views of
      x_full) fetches all source rows in few large calls
    - per chunk: one fused DVE op builds the weighted one-hot
      (iota == cl) * w', w' = w * dinv[row] (computed on device), then a
      PSUM-accumulated 128x128x128 matmul scatters into the target block
    - block tail: road = relu(dinv_t * zp) via one fused mult+max DVE op
    - one big road write, one dma_gather for the trajectory rows
All arithmetic on device; host does sharding, padding, and index layout.
"""

import numpy as np
import ml_dtypes

import concourse.bass as bass
import concourse.bacc as bacc
import concourse.tile as tile
from concourse import mybir, library_config
from concourse.bass_utils import run_bass_kernel_spmd

BF16 = ml_dtypes.bfloat16
P = 128
N, E, D = 100000, 1600000, 128
NCORES = 8
NS = N // NCORES          # 12500 nodes per core
NB = (NS + P - 1) // P    # 98 target blocks per core
NSPAD = NB * P            # 12544
NBANK = 4                 # source banks (row % 4) so gather idxs fit int16
SBB = 7                   # target blocks per gather superblock
NSB = (NB + SBB - 1) // SBB

F32 = mybir.dt.float32
BF = mybir.dt.bfloat16
I16 = mybir.dt.int16

LAST_EXEC_NS = None
LAST_EXEC_PARTS = None
LAST_NCS = None  # (nca, ncb) for offline simulation


def _build_kernel_a(padw):
    nc = bacc.Bacc("TRN2", target_bir_lowering=False, debug=False)
    nfsT = nc.dram_tensor("nfsT", [P, NSPAD], BF, kind="ExternalInput")
    wsb = nc.dram_tensor("wsb", [P, P], BF, kind="ExternalInput")
    wpad = nc.dram_tensor("wpad", [P, NB * padw], F32, kind="ExternalInput")
    x_sh = nc.dram_tensor("x_sh", [P, NSPAD], BF, kind="ExternalOutput")
    dinv_sh = nc.dram_tensor("dinv_sh", [P, NB], F32, kind="ExternalOutput")
    with tile.TileContext(nc) as tc:
        with tc.tile_pool(name="sb", bufs=1) as sb, \
             tc.tile_pool(name="ps", bufs=4, space="PSUM") as ps:
            nf_sb = sb.tile([P, NSPAD], BF)
            nsplit = 7
            step = (NB // nsplit) * P
            for s in range(nsplit):
                lo, hi = s * step, min((s + 1) * step, NSPAD)
                nc.sync.dma_start(nf_sb[:, lo:hi], nfsT[:, lo:hi])
            w_sb = sb.tile([P, P], BF)
            nc.sync.dma_start(w_sb[:], wsb[:])
            wp_sb = sb.tile([P, NB * padw], F32)
            nc.sync.dma_start(wp_sb[:], wpad[:])

            deg = sb.tile([P, NB], F32)
            nc.vector.tensor_reduce(
                out=deg[:],
                in_=wp_sb[:].rearrange("p (b s) -> p b s", s=padw),
                axis=mybir.AxisListType.X,
                op=mybir.AluOpType.add,
            )
            nc.vector.tensor_scalar_add(deg[:], deg[:], 1.0)
            rec = sb.tile([P, NB], F32)
            nc.vector.reciprocal(rec[:], deg[:])
            dinv = sb.tile([P, NB], F32)
            nc.scalar.activation(dinv[:], rec[:], mybir.ActivationFunctionType.Sqrt)
            nc.sync.dma_start(dinv_sh[:], dinv[:])

            xout = sb.tile([P, NSPAD], BF)
            GP = 4   # blocks per PSUM tile (one copy per GP blocks)
            wchunk = 16  # blocks per x_sh writeback
            for g in range(0, NB, GP):
                hi = min(g + GP, NB)
                xp = ps.tile([P, GP * P], F32, tag="xp")
                for j, b in enumerate(range(g, hi)):
                    nc.tensor.matmul(xp[:, j * P:(j + 1) * P],
                                     lhsT=nf_sb[:, b * P:(b + 1) * P],
                                     rhs=w_sb[:], start=True, stop=True)
                nc.vector.tensor_copy(xout[:, g * P:hi * P],
                                      xp[:, :(hi - g) * P])
                if (hi % (wchunk)) == 0 or hi == NB:
                    lo = ((hi - 1) // wchunk) * wchunk * P
                    nc.sync.dma_start(x_sh[:, lo:hi * P], xout[:, lo:hi * P])
    nc.compile()
    return nc


def _schedule(cap):
    """Tight-packed gather schedule, identical across cores.

    cap: [NB][NBANK] slot capacity per (block, bank) group (max over cores).
    Groups pack back-to-back inside each (superblock, bank) call; chunks are
    fixed 128-slot slices of the call, so a chunk can span two adjacent
    blocks (it then feeds one matmul per block, with zero weights masking
    the other block's slots).
    Returns (calls, colbase, novl, slotbase, J2, TOT):
      calls: (sbi, q, nch, slot0) with slot0 the call's global slot base
      colbase/novl: per (b,q) first metadata column and #overlapped chunks
      slotbase: per (b,q) global slot of the group start
      J2: total metadata columns; TOT: total padded slots
    """
    cap = np.asarray(cap)
    calls = []
    colbase = np.zeros((NB, NBANK), np.int64)
    novl = np.zeros((NB, NBANK), np.int64)
    slotbase = np.zeros((NB, NBANK), np.int64)
    col = 0
    slot0 = 0
    for sbi in range(NSB):
        blks = range(sbi * SBB, min((sbi + 1) * SBB, NB))
        for q in range(NBANK):
            caps = [(b, int(cap[b][q])) for b in blks]
            total = sum(c for _, c in caps)
            if total == 0:
                continue
            nch = (total + P - 1) // P
            S = 0
            for b, c in caps:
                slotbase[b][q] = slot0 + S
                if c > 0:
                    colbase[b][q] = col
                    novl[b][q] = (S + c - 1) // P - S // P + 1
                    col += int(novl[b][q])
                S += c
            calls.append((sbi, q, nch, slot0))
            slot0 += nch * P
    return calls, colbase, novl, slotbase, int(col), int(slot0)


def _build_kernel_b(cap, j2):
    """cap: [NB][NBANK] group capacities (identical across cores); j2: output
    gather chunks."""
    call_specs, colbase, novl, slotbase, J, TOT = _schedule(cap)

    nc = bacc.Bacc("TRN2", target_bir_lowering=False, debug=False)
    xfull = nc.dram_tensor("xfull", [N, P], BF, kind="ExternalInput")
    idxs = nc.dram_tensor("idxs", [P, TOT // 16], I16, kind="ExternalInput")
    oidx = nc.dram_tensor("oidx", [P, j2 * P // 16], I16, kind="ExternalInput")
    clp = nc.dram_tensor("clp", [P, J], F32, kind="ExternalInput")
    wdp = nc.dram_tensor("wdp", [P, J], F32, kind="ExternalInput")
    dvp = nc.dram_tensor("dvp", [P, J], F32, kind="ExternalInput")
    dinv_t = nc.dram_tensor("dinv_t", [P, NB], F32, kind="ExternalInput")
    out_packed = nc.dram_tensor("out_packed", [P, j2 * P], BF,
                                kind="ExternalOutput")

    # bank view of xfull: rows r with r % NBANK == q, idx = r // NBANK
    xview = xfull[:].rearrange("(n f) d -> n f d", f=NBANK)

    with tile.TileContext(nc) as tc:
        with tc.tile_pool(name="sb", bufs=1) as sb, \
             tc.tile_pool(name="gp", bufs=2) as gp, \
             tc.tile_pool(name="op", bufs=12) as op_, \
             tc.tile_pool(name="ps", bufs=4, space="PSUM") as ps:
            nc.gpsimd.load_library(library_config.mlp)

            # idx loads split per superblock so the first gather starts early
            idx_sb = sb.tile([P, TOT // 16], I16)
            sb_col = []  # (col0, col1) per superblock
            c0 = 0
            for sbi in range(NSB):
                ncols = 8 * sum(nch for s, q, nch, _ in call_specs if s == sbi)
                sb_col.append((c0, c0 + ncols))
                if ncols:
                    nc.sync.dma_start(idx_sb[:, c0:c0 + ncols],
                                      idxs[:, c0:c0 + ncols])
                c0 += ncols
            assert c0 == TOT // 16
            oix_sb = sb.tile([P, j2 * P // 16], I16)
            nc.sync.dma_start(oix_sb[:], oidx[:])
            cl_sb = sb.tile([P, J], F32)
            nc.sync.dma_start(cl_sb[:], clp[:])
            wd_sb = sb.tile([P, J], F32)
            nc.sync.dma_start(wd_sb[:], wdp[:])
            dv_sb = sb.tile([P, J], F32)
            nc.sync.dma_start(dv_sb[:], dvp[:])
            dt_sb = sb.tile([P, NB], F32)
            nc.sync.dma_start(dt_sb[:], dinv_t[:])

            # w' = w * dinv[row], one big fused op
            wf_sb = sb.tile([P, J], F32)
            nc.vector.tensor_tensor(out=wf_sb[:], in0=wd_sb[:], in1=dv_sb[:],
                                    op=mybir.AluOpType.mult)

            iota_i = sb.tile([P, P], mybir.dt.int32)
            nc.gpsimd.iota(iota_i[:], pattern=[[1, P]], channel_multiplier=0)
            iota_f = sb.tile([P, P], F32)
            nc.vector.tensor_copy(iota_f[:], iota_i[:])
            iota_bf = sb.tile([P, P], BF)
            nc.vector.tensor_copy(iota_bf[:], iota_f[:])

            road_sb = sb.tile([P, NSPAD], BF)

            ci = 0  # call index
            for sbi in range(NSB):
                blks = range(sbi * SBB, min((sbi + 1) * SBB, NB))
                gts = {}
                for q in range(NBANK):
                    if ci < len(call_specs) and call_specs[ci][0] == sbi \
                            and call_specs[ci][1] == q:
                        _, _, nch, slot0 = call_specs[ci]
                        ci += 1
                        gt = gp.tile([P, nch * P], BF, tag=f"g{q}")
                        nc.gpsimd.dma_gather(
                            gt[:].rearrange("p (j d) -> p j d", d=P),
                            xview[:, q, :],
                            idx_sb[:, slot0 // 16:slot0 // 16 + nch * 8],
                            nch * P, nch * P, P, elem_step=NBANK * P,
                            single_packet=False)
                        gts[q] = (gt, slot0)
                for b in blks:
                    tot = int(novl[b].sum())
                    zp = ps.tile([P, P], F32, tag="zp")
                    ji = 0
                    for q in range(NBANK):
                        no = int(novl[b][q])
                        if no == 0:
                            continue
                        gt, slot0 = gts[q]
                        ch0 = (int(slotbase[b][q]) - slot0) // P
                        for lc in range(no):
                            col = int(colbase[b][q]) + lc
                            c = ch0 + lc
                            ohw = op_.tile([P, P], BF, tag="oh")
                            nc.vector.tensor_scalar(
                                ohw[:], iota_bf[:], cl_sb[:, col:col + 1],
                                wf_sb[:, col:col + 1],
                                mybir.AluOpType.is_equal, mybir.AluOpType.mult)
                            nc.tensor.matmul(
                                zp[:], lhsT=ohw[:],
                                rhs=gt[:, c * P:(c + 1) * P],
                                start=(ji == 0), stop=(ji == tot - 1))
                            ji += 1
                    nc.vector.tensor_scalar(
                        road_sb[:, b * P:(b + 1) * P], zp[:],
                        dt_sb[:, b:b + 1], 0.0,
                        mybir.AluOpType.mult, mybir.AluOpType.max)

            # trajectory gather straight from SBUF road (transposed output):
            # idx v -> partition v % 128, rank (block) v // 128
            og = sb.tile([P, j2 * P], BF)
            nc.gpsimd.dma_gather(
                og[:].rearrange("p (c n) -> p c n", c=1),
                road_sb[:], oix_sb[:], j2 * P, j2 * P, P,
                transpose=True, single_packet=False,
                sbuf_tokens_per_rank=P,
                sbuf_free_dim_per_rank=P * 2,
                sbuf_byte_offset=0)
            nc.sync.dma_start(out_packed[:], og[:])
    nc.compile()
    return nc


def kernel(**inputs):
    traj = np.asarray(inputs["traj_seqs"])[..., 0].astype(np.int64)
    seq_len = np.asarray(inputs["seq_len"]).astype(np.int64)
    nf = np.asarray(inputs["node_feat"], dtype=np.float32)
    ei = np.asarray(inputs["edge_index"]).astype(np.int64)
    ef = np.asarray(inputs["edge_feat"], dtype=np.float32)
    W = np.asarray(inputs["W"], dtype=np.float32)
    b = np.asarray(inputs["b"], dtype=np.float32)
    assert np.all(b == 0.0), "nonzero bias not wired into device path"

    row, col = ei[0], ei[1]
    owner = col // NS

    # ---------- per-core edge sets (self-loops as real edges) ----------
    loops_c = np.arange(NS, dtype=np.int64)
    core_edges = []
    cnts = np.zeros((NCORES, NB, NBANK), np.int64)
    for k in range(NCORES):
        m = owner == k
        ck = np.concatenate([col[m] - k * NS, loops_c])
        rk = np.concatenate([row[m], k * NS + loops_c])
        wk = np.concatenate([ef[m], np.ones(NS, np.float32)])
        bq = (ck // P) * NBANK + (rk % NBANK)
        srt = np.argsort(bq, kind="stable")
        ck, rk, wk, bq = ck[srt], rk[srt], wk[srt], bq[srt]
        np.add.at(cnts, (k, bq // NBANK, bq % NBANK), 1)
        core_edges.append((ck, rk, wk, bq))

    cap = cnts.max(axis=0)  # [NB, NBANK] tight group capacities
    _, colbase, novl, slotbase, J, TOT = _schedule(cap)

    # trajectory selection
    flat = traj.reshape(-1)
    L = traj.shape[1]
    posmask = (np.arange(L)[None, :] < seq_len[:, None]).reshape(-1)
    oo = flat // NS
    sels = [np.where((oo == k) & posmask)[0] for k in range(NCORES)]
    j2 = max(1, int(np.ceil(max(len(s) for s in sels) / P)))

    # ---------- launch A ----------
    padw = 1
    for k in range(NCORES):
        m = owner == k
        c_loc = col[m] - k * NS
        padw = max(padw, int(np.bincount(c_loc, minlength=NS).max()))

    nf_bf = nf.astype(BF16)
    W_bf = np.ascontiguousarray(W.astype(BF16))
    in_maps_a = []
    for k in range(NCORES):
        m = owner == k
        c_loc = col[m] - k * NS
        w_loc = ef[m]
        cnt = np.bincount(c_loc, minlength=NS)
        starts = np.zeros(NS, np.int64)
        np.cumsum(cnt[:-1], out=starts[1:])
        srt = np.argsort(c_loc, kind="stable")
        cs, ws = c_loc[srt], w_loc[srt]
        posin = np.arange(len(cs)) - starts[cs]
        arr = np.zeros((NSPAD, padw), np.float32)
        arr[cs, posin] = ws
        wpad = np.ascontiguousarray(
            arr.reshape(NB, P, padw).transpose(1, 0, 2).reshape(P, NB * padw))
        nfsT = np.zeros((P, NSPAD), BF16)
        nfsT[:, :NS] = nf_bf[k * NS:(k + 1) * NS].T
        in_maps_a.append({"nfsT": nfsT, "wsb": W_bf, "wpad": wpad})

    nca = _build_kernel_a(padw)
    ra = run_bass_kernel_spmd(nca, in_maps_a, core_ids=list(range(NCORES)))

    # host reassembly: x_full rows, dinv_full
    x_full = np.zeros((N, P), BF16)
    dinv_full = np.zeros(N, np.float32)
    for k in range(NCORES):
        xs = ra.results[k]["x_sh"]          # [128, NSPAD]
        xr = xs.reshape(P, NB, P).transpose(1, 0, 2).reshape(NSPAD, P)
        x_full[k * NS:(k + 1) * NS] = xr[:NS]
        ds = ra.results[k]["dinv_sh"]       # [128, NB]
        dr = ds.T.reshape(NSPAD)
        dinv_full[k * NS:(k + 1) * NS] = dr[:NS]
    x_full = np.ascontiguousarray(x_full)

    # ---------- launch B ----------
    in_maps_b = []
    for k in range(NCORES):
        ck, rk, wk, bq = core_edges[k]
        bqcnt = np.bincount(bq, minlength=NB * NBANK).reshape(NB, NBANK)
        gstart = np.zeros(NB * NBANK, np.int64)
        np.cumsum(bqcnt.reshape(-1)[:-1], out=gstart[1:])
        pos = np.arange(len(ck)) - gstart[bq]
        sbase = slotbase[bq // NBANK, bq % NBANK]
        f = sbase + pos                       # global flat slot
        col = colbase[bq // NBANK, bq % NBANK] + (f // P - sbase // P)
        par = f % P

        clp = np.zeros((P, J), np.float32)
        wdp = np.zeros((P, J), np.float32)
        dvp = np.zeros((P, J), np.float32)
        clp[par, col] = (ck % P).astype(np.float32)
        wdp[par, col] = wk
        dvp[par, col] = dinv_full[rk]

        idx_arr = np.zeros((16, TOT // 16), np.int16)
        idx_arr[f % 16, f // 16] = (rk // NBANK).astype(np.int16)
        idx_t = np.tile(idx_arr, (8, 1))      # replicate to 128 partitions

        lv = (flat[sels[k]] - k * NS).astype(np.int16)
        oarr = np.zeros((16, j2 * P // 16), np.int16)
        fo = np.arange(len(lv))
        oarr[fo % 16, fo // 16] = lv
        oidx_t = np.tile(oarr, (8, 1))

        in_maps_b.append({
            "xfull": x_full, "idxs": idx_t, "oidx": oidx_t,
            "clp": clp, "wdp": wdp, "dvp": dvp,
            "dinv_t": ra.results[k]["dinv_sh"],
        })

    ncb = _build_kernel_b(cap, j2)
    rb = run_bass_kernel_spmd(ncb, in_maps_b, core_ids=list(range(NCORES)))

    global LAST_EXEC_NS, LAST_EXEC_PARTS, LAST_NCS
    LAST_NCS = (nca, ncb)
    LAST_EXEC_PARTS = (ra.exec_time_ns, rb.exec_time_ns)
    if ra.exec_time_ns and rb.exec_time_ns:
        LAST_EXEC_NS = ra.exec_time_ns + rb.exec_time_ns

    out = np.zeros((flat.shape[0], D), np.float32)
    for k in range(NCORES):
        if len(sels[k]):
            out[sels[k]] = rb.results[k]["out_packed"][:, :len(sels[k])].T.astype(
                np.float32)
    return out.reshape(traj.shape[0], L, D)


# revision 30
# speedup vs baseline: 1.1600x; 1.0729x over previous
"""Trainium2 Bass kernel for nn_LocationEmbedding (GCN scatter-add + trajectory gather).

Strategy (8 NeuronCores, SPMD, two launches):
  Launch A (per core, owns nodes [k*12500, (k+1)*12500)):
    - deg via segmented reduce of host-padded edge weights, dinv = rsqrt(deg+1)
    - x = node_feat @ W per 128-row block (bf16 matmuls)
    Host assembles x_full [100000,128] bf16 and dinv_full [100000] f32.
  Launch B (per core, target-sharded edges incl. self-loops as real edges):
    - edges grouped by (target block, source bank = row % 4), padded to
      128-slot chunks; bulk dma_gather (int16 idxs, strided bank views of
      x_full) fetches all source rows in few large calls
    - per chunk: one fused DVE op builds the weighted one-hot
      (iota == cl) * w', w' = w * dinv[row] (computed on device), then a
      PSUM-accumulated 128x128x128 matmul scatters into the target block
    - block tail: road = relu(dinv_t * zp) via one fused mult+max DVE op
    - one big road write, one dma_gather for the trajectory rows
All arithmetic on device; host does sharding, padding, and index layout.
"""

import numpy as np
import ml_dtypes

import concourse.bass as bass
import concourse.bacc as bacc
import concourse.tile as tile
from concourse import mybir, library_config
from concourse.bass_utils import run_bass_kernel_spmd

BF16 = ml_dtypes.bfloat16
P = 128
N, E, D = 100000, 1600000, 128
NCORES = 8
NS = N // NCORES          # 12500 nodes per core
NB = (NS + P - 1) // P    # 98 target blocks per core
NSPAD = NB * P            # 12544
NBANK = 4                 # source banks (row % 4) so gather idxs fit int16
# superblock sizes (blocks per gather round); small first/last shrink the
# pipeline head/tail
SB_SIZES = [4] + [8] * 11 + [4, 2]
assert sum(SB_SIZES) == NB
SB_BLKS = []
_b0 = 0
for _s in SB_SIZES:
    SB_BLKS.append(range(_b0, _b0 + _s))
    _b0 += _s
NSB = len(SB_BLKS)
IDXREP = 2                # idx tiles replicated to 2x16 partitions

F32 = mybir.dt.float32
BF = mybir.dt.bfloat16
I16 = mybir.dt.int16

LAST_EXEC_NS = None
LAST_EXEC_PARTS = None
LAST_NCS = None  # (nca, ncb) for offline simulation


def _build_kernel_a(padw):
    """deg/dinv only: dinv = rsqrt(1 + segmented-sum of edge weights)."""
    nc = bacc.Bacc("TRN2", target_bir_lowering=False, debug=False)
    wpad = nc.dram_tensor("wpad", [P, NB * padw], F32, kind="ExternalInput")
    dinv_sh = nc.dram_tensor("dinv_sh", [P, NB], F32, kind="ExternalOutput")
    with tile.TileContext(nc) as tc:
        with tc.tile_pool(name="sb", bufs=1) as sb:
            wp_sb = sb.tile([P, NB * padw], F32)
            nc.sync.dma_start(wp_sb[:], wpad[:])
            deg = sb.tile([P, NB], F32)
            nc.vector.tensor_reduce(
                out=deg[:],
                in_=wp_sb[:].rearrange("p (b s) -> p b s", s=padw),
                axis=mybir.AxisListType.X,
                op=mybir.AluOpType.add,
            )
            nc.vector.tensor_scalar_add(deg[:], deg[:], 1.0)
            rec = sb.tile([P, NB], F32)
            nc.vector.reciprocal(rec[:], deg[:])
            dinv = sb.tile([P, NB], F32)
            nc.scalar.activation(dinv[:], rec[:], mybir.ActivationFunctionType.Sqrt)
            nc.sync.dma_start(dinv_sh[:], dinv[:])
    nc.compile()
    return nc


def _schedule(cap):
    """Tight-packed gather schedule, identical across cores.

    cap: [NB][NBANK] slot capacity per (block, bank) group (max over cores).
    Groups pack back-to-back inside each (superblock, bank) call; chunks are
    fixed 128-slot slices of the call, so a chunk can span two adjacent
    blocks (it then feeds one matmul per block, with zero weights masking
    the other block's slots).
    Returns (calls, colbase, novl, slotbase, J2, TOT):
      calls: (sbi, q, nch, slot0) with slot0 the call's global slot base
      colbase/novl: per (b,q) first metadata column and #overlapped chunks
      slotbase: per (b,q) global slot of the group start
      J2: total metadata columns; TOT: total padded slots
    """
    cap = np.asarray(cap)
    calls = []
    colbase = np.zeros((NB, NBANK), np.int64)
    novl = np.zeros((NB, NBANK), np.int64)
    slotbase = np.zeros((NB, NBANK), np.int64)
    col = 0
    slot0 = 0
    for sbi in range(NSB):
        blks = SB_BLKS[sbi]
        for q in range(NBANK):
            caps = [(b, int(cap[b][q])) for b in blks]
            total = sum(c for _, c in caps)
            if total == 0:
                continue
            nch = (total + P - 1) // P
            S = 0
            for b, c in caps:
                slotbase[b][q] = slot0 + S
                if c > 0:
                    colbase[b][q] = col
                    novl[b][q] = (S + c - 1) // P - S // P + 1
                    col += int(novl[b][q])
                S += c
            calls.append((sbi, q, nch, slot0))
            slot0 += nch * P
    return calls, colbase, novl, slotbase, int(col), int(slot0)


def _build_kernel_b(cap, j2):
    """cap: [NB][NBANK] group capacities (identical across cores); j2: output
    gather chunks."""
    call_specs, colbase, novl, slotbase, J, TOT = _schedule(cap)

    nc = bacc.Bacc("TRN2", target_bir_lowering=False, debug=False)
    xfull = nc.dram_tensor("xfull", [N, P], BF, kind="ExternalInput")
    wsb = nc.dram_tensor("wsb", [P, P], BF, kind="ExternalInput")
    idxs = nc.dram_tensor("idxs", [IDXREP * 16, TOT // 16], I16,
                          kind="ExternalInput")
    oidx = nc.dram_tensor("oidx", [IDXREP * 16, j2 * P // 16], I16,
                          kind="ExternalInput")
    clp = nc.dram_tensor("clp", [P, J], F32, kind="ExternalInput")
    wdp = nc.dram_tensor("wdp", [P, J], BF, kind="ExternalInput")
    dvp = nc.dram_tensor("dvp", [P, J], BF, kind="ExternalInput")
    dinv_t = nc.dram_tensor("dinv_t", [P, NB], F32, kind="ExternalInput")
    out_packed = nc.dram_tensor("out_packed", [P, j2 * P], BF,
                                kind="ExternalOutput")

    # bank view of xfull: rows r with r % NBANK == q, idx = r // NBANK
    xview = xfull[:].rearrange("(n f) d -> n f d", f=NBANK)

    with tile.TileContext(nc) as tc:
        with tc.tile_pool(name="sb", bufs=1) as sb, \
             tc.tile_pool(name="gp", bufs=2) as gp, \
             tc.tile_pool(name="op", bufs=12) as op_, \
             tc.tile_pool(name="ps", bufs=4, space="PSUM") as ps:
            nc.gpsimd.load_library(library_config.mlp)

            # idx loads split per superblock so the first gather starts early
            idx_sb = sb.tile([IDXREP * 16, TOT // 16], I16)
            sb_col = []  # (col0, col1) per superblock
            c0 = 0
            for sbi in range(NSB):
                ncols = 8 * sum(nch for s, q, nch, _ in call_specs if s == sbi)
                sb_col.append((c0, c0 + ncols))
                if ncols:
                    nc.sync.dma_start(idx_sb[:, c0:c0 + ncols],
                                      idxs[:, c0:c0 + ncols])
                c0 += ncols
            assert c0 == TOT // 16
            oix_sb = sb.tile([IDXREP * 16, j2 * P // 16], I16)
            nc.sync.dma_start(oix_sb[:], oidx[:])
            cl_sb = sb.tile([P, J], F32)
            nc.sync.dma_start(cl_sb[:], clp[:])
            wd_sb = sb.tile([P, J], BF)
            nc.sync.dma_start(wd_sb[:], wdp[:])
            dv_sb = sb.tile([P, J], BF)
            nc.sync.dma_start(dv_sb[:], dvp[:])
            dt_sb = sb.tile([P, NB], F32)
            nc.sync.dma_start(dt_sb[:], dinv_t[:])
            w_sb = sb.tile([P, P], BF)
            nc.sync.dma_start(w_sb[:], wsb[:])

            # w' = w * dinv[row], one big fused op
            wf_sb = sb.tile([P, J], F32)
            nc.vector.tensor_tensor(out=wf_sb[:], in0=wd_sb[:], in1=dv_sb[:],
                                    op=mybir.AluOpType.mult)

            iota_i = sb.tile([P, P], mybir.dt.int32)
            nc.gpsimd.iota(iota_i[:], pattern=[[1, P]], channel_multiplier=0)
            iota_f = sb.tile([P, P], F32)
            nc.vector.tensor_copy(iota_f[:], iota_i[:])
            iota_bf = sb.tile([P, P], BF)
            nc.vector.tensor_copy(iota_bf[:], iota_f[:])

            road_sb = sb.tile([P, NSPAD], BF)

            ci = 0  # call index
            for sbi in range(NSB):
                blks = SB_BLKS[sbi]
                gts = {}
                for q in range(NBANK):
                    if ci < len(call_specs) and call_specs[ci][0] == sbi \
                            and call_specs[ci][1] == q:
                        _, _, nch, slot0 = call_specs[ci]
                        ci += 1
                        gt = gp.tile([P, nch * P], BF, tag=f"g{q}")
                        nc.gpsimd.dma_gather(
                            gt[:].rearrange("p (j d) -> p j d", d=P),
                            xview[:, q, :],
                            idx_sb[:, slot0 // 16:slot0 // 16 + nch * 8],
                            nch * P, nch * P, P, elem_step=NBANK * P,
                            single_packet=False)
                        gts[q] = (gt, slot0)
                for b in blks:
                    tot = int(novl[b].sum())
                    zp = ps.tile([P, P], F32, tag="zp")
                    ji = 0
                    for q in range(NBANK):
                        no = int(novl[b][q])
                        if no == 0:
                            continue
                        gt, slot0 = gts[q]
                        ch0 = (int(slotbase[b][q]) - slot0) // P
                        for lc in range(no):
                            col = int(colbase[b][q]) + lc
                            c = ch0 + lc
                            ohw = op_.tile([P, P], BF, tag="oh")
                            nc.vector.tensor_scalar(
                                ohw[:], iota_bf[:], cl_sb[:, col:col + 1],
                                wf_sb[:, col:col + 1],
                                mybir.AluOpType.is_equal, mybir.AluOpType.mult)
                            # zp[d, c] += sum_p gt[p, d] * ohw[p, c]   (s^T)
                            nc.tensor.matmul(
                                zp[:], lhsT=gt[:, c * P:(c + 1) * P],
                                rhs=ohw[:],
                                start=(ji == 0), stop=(ji == tot - 1))
                            ji += 1
                    sT = op_.tile([P, P], BF, tag="sT")
                    nc.scalar.activation(sT[:], zp[:],
                                         mybir.ActivationFunctionType.Copy)
                    tp = ps.tile([P, P], F32, tag="tp")
                    nc.tensor.matmul(tp[:], lhsT=sT[:], rhs=w_sb[:],
                                     start=True, stop=True)
                    nc.scalar.activation(
                        road_sb[:, b * P:(b + 1) * P], tp[:],
                        mybir.ActivationFunctionType.Relu,
                        scale=dt_sb[:, b:b + 1])

            # trajectory gather straight from SBUF road (transposed output):
            # idx v -> partition v % 128, rank (block) v // 128
            og = sb.tile([P, j2 * P], BF)
            nc.gpsimd.dma_gather(
                og[:].rearrange("p (c n) -> p c n", c=1),
                road_sb[:], oix_sb[:], j2 * P, j2 * P, P,
                transpose=True, single_packet=False,
                sbuf_tokens_per_rank=P,
                sbuf_free_dim_per_rank=P * 2,
                sbuf_byte_offset=0)
            nc.sync.dma_start(out_packed[:], og[:])
    nc.compile()
    return nc


def kernel(**inputs):
    traj = np.asarray(inputs["traj_seqs"])[..., 0].astype(np.int64)
    seq_len = np.asarray(inputs["seq_len"]).astype(np.int64)
    nf = np.asarray(inputs["node_feat"], dtype=np.float32)
    ei = np.asarray(inputs["edge_index"]).astype(np.int64)
    ef = np.asarray(inputs["edge_feat"], dtype=np.float32)
    W = np.asarray(inputs["W"], dtype=np.float32)
    b = np.asarray(inputs["b"], dtype=np.float32)
    assert np.all(b == 0.0), "nonzero bias not wired into device path"

    row, col = ei[0], ei[1]
    owner = col // NS

    # ---------- per-core edge sets (self-loops as real edges) ----------
    loops_c = np.arange(NS, dtype=np.int64)
    core_edges = []
    cnts = np.zeros((NCORES, NB, NBANK), np.int64)
    for k in range(NCORES):
        m = owner == k
        ck = np.concatenate([col[m] - k * NS, loops_c])
        rk = np.concatenate([row[m], k * NS + loops_c])
        wk = np.concatenate([ef[m], np.ones(NS, np.float32)])
        bq = (ck // P) * NBANK + (rk % NBANK)
        srt = np.argsort(bq, kind="stable")
        ck, rk, wk, bq = ck[srt], rk[srt], wk[srt], bq[srt]
        np.add.at(cnts, (k, bq // NBANK, bq % NBANK), 1)
        core_edges.append((ck, rk, wk, bq))

    cap = cnts.max(axis=0)  # [NB, NBANK] tight group capacities
    _, colbase, novl, slotbase, J, TOT = _schedule(cap)

    # trajectory selection
    flat = traj.reshape(-1)
    L = traj.shape[1]
    posmask = (np.arange(L)[None, :] < seq_len[:, None]).reshape(-1)
    oo = flat // NS
    sels = [np.where((oo == k) & posmask)[0] for k in range(NCORES)]
    j2 = max(1, int(np.ceil(max(len(s) for s in sels) / P)))

    # ---------- launch A (deg/dinv only) ----------
    padw = 1
    for k in range(NCORES):
        m = owner == k
        c_loc = col[m] - k * NS
        padw = max(padw, int(np.bincount(c_loc, minlength=NS).max()))

    x_full = np.ascontiguousarray(nf.astype(BF16))   # gather source (raw feats)
    W_bf = np.ascontiguousarray(W.astype(BF16))
    in_maps_a = []
    for k in range(NCORES):
        m = owner == k
        c_loc = col[m] - k * NS
        w_loc = ef[m]
        cnt = np.bincount(c_loc, minlength=NS)
        starts = np.zeros(NS, np.int64)
        np.cumsum(cnt[:-1], out=starts[1:])
        srt = np.argsort(c_loc, kind="stable")
        cs, ws = c_loc[srt], w_loc[srt]
        posin = np.arange(len(cs)) - starts[cs]
        arr = np.zeros((NSPAD, padw), np.float32)
        arr[cs, posin] = ws
        wpad = np.ascontiguousarray(
            arr.reshape(NB, P, padw).transpose(1, 0, 2).reshape(P, NB * padw))
        in_maps_a.append({"wpad": wpad})

    nca = _build_kernel_a(padw)
    ra = run_bass_kernel_spmd(nca, in_maps_a, core_ids=list(range(NCORES)))

    dinv_full = np.zeros(N, np.float32)
    for k in range(NCORES):
        ds = ra.results[k]["dinv_sh"]       # [128, NB]
        dr = ds.T.reshape(NSPAD)
        dinv_full[k * NS:(k + 1) * NS] = dr[:NS]

    # ---------- launch B ----------
    in_maps_b = []
    for k in range(NCORES):
        ck, rk, wk, bq = core_edges[k]
        bqcnt = np.bincount(bq, minlength=NB * NBANK).reshape(NB, NBANK)
        gstart = np.zeros(NB * NBANK, np.int64)
        np.cumsum(bqcnt.reshape(-1)[:-1], out=gstart[1:])
        pos = np.arange(len(ck)) - gstart[bq]
        sbase = slotbase[bq // NBANK, bq % NBANK]
        f = sbase + pos                       # global flat slot
        col = colbase[bq // NBANK, bq % NBANK] + (f // P - sbase // P)
        par = f % P

        clp = np.zeros((P, J), np.float32)
        wdp = np.zeros((P, J), BF16)
        dvp = np.zeros((P, J), BF16)
        clp[par, col] = (ck % P).astype(np.float32)
        wdp[par, col] = wk.astype(BF16)
        dvp[par, col] = dinv_full[rk].astype(BF16)

        idx_arr = np.zeros((16, TOT // 16), np.int16)
        idx_arr[f % 16, f // 16] = (rk // NBANK).astype(np.int16)
        idx_t = np.tile(idx_arr, (IDXREP, 1))

        lv = (flat[sels[k]] - k * NS).astype(np.int16)
        oarr = np.zeros((16, j2 * P // 16), np.int16)
        fo = np.arange(len(lv))
        oarr[fo % 16, fo // 16] = lv
        oidx_t = np.tile(oarr, (IDXREP, 1))

        in_maps_b.append({
            "xfull": x_full, "wsb": W_bf, "idxs": idx_t, "oidx": oidx_t,
            "clp": clp, "wdp": wdp, "dvp": dvp,
            "dinv_t": ra.results[k]["dinv_sh"],
        })

    ncb = _build_kernel_b(cap, j2)
    rb = run_bass_kernel_spmd(ncb, in_maps_b, core_ids=list(range(NCORES)))

    global LAST_EXEC_NS, LAST_EXEC_PARTS, LAST_NCS
    LAST_NCS = (nca, ncb)
    LAST_EXEC_PARTS = (ra.exec_time_ns, rb.exec_time_ns)
    if ra.exec_time_ns and rb.exec_time_ns:
        LAST_EXEC_NS = ra.exec_time_ns + rb.exec_time_ns

    out = np.zeros((flat.shape[0], D), np.float32)
    for k in range(NCORES):
        if len(sels[k]):
            out[sels[k]] = rb.results[k]["out_packed"][:, :len(sels[k])].T.astype(
                np.float32)
    return out.reshape(traj.shape[0], L, D)


# revision 49
# speedup vs baseline: 1.2445x; 1.0728x over previous
"""Trainium2 Bass kernel for nn_LocationEmbedding (GCN scatter-add + trajectory gather).

Strategy (8 NeuronCores, SPMD, two launches):
  Launch A (per core, owns nodes [k*12500, (k+1)*12500)):
    deg via segmented reduce of host-padded bf16 edge weights,
    dinv = rsqrt(deg + 1). Host assembles dinv_full [100000] f32.
  Launch B (per core, target-sharded edges; self-loops handled densely):
    - edges grouped by (target 128-block, source bank = row % 4; 4 banks so
      gather idxs fit int16), tight-packed into per-(superblock, bank)
      dma_gather calls over strided bank views of bf16 node features; a
      128-slot chunk may straddle two blocks (one matmul per block, zero
      weights masking foreign slots)
    - per chunk column: one fused DVE op builds the weighted one-hot
      (iota == cl) * w', w' = w * dinv[row] (computed on device); matmul
      (lhsT=gathered, rhs=one-hot) accumulates s^T per block in PSUM,
      bank-major so the PE never stalls on a later bank's gather; each
      block's accumulator owns a full PSUM bank
    - self-loop term added per block via a dense diagonal matmul from a
      locally-loaded feature tile (no gather slots spent on self-loops)
    - block tail on the Activation engine: sT = copy(z^T), t = sT @ W,
      road = Relu(t, scale=dinv_t)
    - trajectory rows fetched by an SBUF-source transpose dma_gather from
      the road tile; output returned transposed, host transposes back
    All per-superblock loads (idxs, cl/w/dinv metadata, local features)
    interleave with the gather stream instead of serializing at the head.
All arithmetic on device; host does sharding, padding, and index layout.
"""

import numpy as np
import ml_dtypes

import concourse.bass as bass
import concourse.bacc as bacc
import concourse.tile as tile
from concourse import mybir, library_config
from concourse.bass_utils import run_bass_kernel_spmd
from concourse.masks import make_identity

BF16 = ml_dtypes.bfloat16
P = 128
N, E, D = 100000, 1600000, 128
NCORES = 8
NS = N // NCORES          # 12500 nodes per core
NB = (NS + P - 1) // P    # 98 target blocks per core
NSPAD = NB * P            # 12544
NBANK = 4                 # source banks (row % 4) so gather idxs fit int16
# superblock sizes (blocks per gather round); small first/last shrink the
# pipeline head/tail; max 6 so each block's PSUM accumulator owns a bank
SB_SIZES = [4] + [6] * 15 + [4]
assert sum(SB_SIZES) == NB
SB_BLKS = []
_b0 = 0
for _s in SB_SIZES:
    SB_BLKS.append(range(_b0, _b0 + _s))
    _b0 += _s
NSB = len(SB_BLKS)
IDXREP = 2                # idx tiles replicated to 2x16 partitions

F32 = mybir.dt.float32
BF = mybir.dt.bfloat16
I16 = mybir.dt.int16

LAST_EXEC_NS = None
LAST_EXEC_PARTS = None
LAST_NCS = None  # (nca, ncb) for offline simulation


def _build_kernel_a(padw):
    """deg/dinv only: dinv = rsqrt(1 + segmented-sum of edge weights)."""
    nc = bacc.Bacc("TRN2", target_bir_lowering=False, debug=False)
    wpad = nc.dram_tensor("wpad", [P, NB * padw], BF, kind="ExternalInput")
    dinv_sh = nc.dram_tensor("dinv_sh", [P, NB], F32, kind="ExternalOutput")
    with tile.TileContext(nc) as tc:
        with tc.tile_pool(name="sb", bufs=1) as sb:
            wp_sb = sb.tile([P, NB * padw], BF)
            deg = sb.tile([P, NB], F32)
            half = (NB // 2) * padw
            for lo, hi, b0, b1 in ((0, half, 0, NB // 2),
                                   (half, NB * padw, NB // 2, NB)):
                nc.sync.dma_start(wp_sb[:, lo:hi], wpad[:, lo:hi])
                nc.vector.tensor_reduce(
                    out=deg[:, b0:b1],
                    in_=wp_sb[:, lo:hi].rearrange("p (b s) -> p b s", s=padw),
                    axis=mybir.AxisListType.X,
                    op=mybir.AluOpType.add,
                )
            nc.vector.tensor_scalar_add(deg[:], deg[:], 1.0)
            rec = sb.tile([P, NB], F32)
            nc.vector.reciprocal(rec[:], deg[:])
            dinv = sb.tile([P, NB], F32)
            nc.scalar.activation(dinv[:], rec[:], mybir.ActivationFunctionType.Sqrt)
            nc.sync.dma_start(dinv_sh[:], dinv[:])
    nc.compile()
    return nc


def _schedule(cap):
    """Tight-packed gather schedule, identical across cores.

    cap: [NB][NBANK] slot capacity per (block, bank) group (max over cores).
    Groups pack back-to-back inside each (superblock, bank) call; chunks are
    fixed 128-slot slices of the call, so a chunk can span two adjacent
    blocks (it then feeds one matmul per block, with zero weights masking
    the other block's slots).
    Returns (calls, colbase, novl, slotbase, J2, TOT):
      calls: (sbi, q, nch, slot0) with slot0 the call's global slot base
      colbase/novl: per (b,q) first metadata column and #overlapped chunks
      slotbase: per (b,q) global slot of the group start
      J2: total metadata columns; TOT: total padded slots
    """
    cap = np.asarray(cap)
    calls = []  # (sbi, q, nch, slot0, exact_slots)
    colbase = np.zeros((NB, NBANK), np.int64)
    novl = np.zeros((NB, NBANK), np.int64)
    slotbase = np.zeros((NB, NBANK), np.int64)
    sbcols = []
    col = 0
    slot0 = 0
    for sbi in range(NSB):
        blks = SB_BLKS[sbi]
        col_lo = col
        for q in range(NBANK):
            caps = [(b, int(cap[b][q])) for b in blks]
            total = sum(c for _, c in caps)
            if total == 0:
                continue
            nch = (total + P - 1) // P
            exact = total
            S = 0
            for b, c in caps:
                slotbase[b][q] = slot0 + S
                if c > 0:
                    colbase[b][q] = col
                    novl[b][q] = (S + c - 1) // P - S // P + 1
                    col += int(novl[b][q])
                S += c
            calls.append((sbi, q, nch, slot0, exact))
            slot0 += nch * P
        sbcols.append((col_lo, col))
    return calls, colbase, novl, slotbase, int(col), int(slot0), sbcols


SPLIT_B = NB - SB_SIZES[-1]   # trajectory rows below this block gather early


def _build_kernel_b(cap, j2a, j2b):
    """cap: [NB][NBANK] group capacities (identical across cores); j2a/j2b:
    output gather chunks for road rows below/above SPLIT_B*128."""
    j2 = j2a + j2b
    call_specs, colbase, novl, slotbase, J, TOT, sbcols = _schedule(cap)

    nc = bacc.Bacc("TRN2", target_bir_lowering=False, debug=False)
    xfull = nc.dram_tensor("xfull", [N, P], BF, kind="ExternalInput")
    wsb = nc.dram_tensor("wsb", [P, P], BF, kind="ExternalInput")
    idxs = nc.dram_tensor("idxs", [IDXREP * 16, TOT // 16], I16,
                          kind="ExternalInput")
    oidx = nc.dram_tensor("oidx", [IDXREP * 16, j2 * P // 16], I16,
                          kind="ExternalInput")
    clp = nc.dram_tensor("clp", [P, J], F32, kind="ExternalInput")
    wdp = nc.dram_tensor("wdp", [P, J], BF, kind="ExternalInput")
    dvp = nc.dram_tensor("dvp", [P, J], BF, kind="ExternalInput")
    dinv_t = nc.dram_tensor("dinv_t", [P, NB], F32, kind="ExternalInput")
    nfloc = nc.dram_tensor("nfloc", [P, NSPAD], BF, kind="ExternalInput")
    out_packed = nc.dram_tensor("out_packed", [P, j2 * P], BF,
                                kind="ExternalOutput")

    # bank view of xfull: rows r with r % NBANK == q, idx = r // NBANK
    xview = xfull[:].rearrange("(n f) d -> n f d", f=NBANK)

    with tile.TileContext(nc) as tc:
        with tc.tile_pool(name="sb", bufs=1) as sb, \
             tc.tile_pool(name="gp", bufs=2) as gp, \
             tc.tile_pool(name="op", bufs=20) as op_, \
             tc.tile_pool(name="psz", bufs=1, space="PSUM") as psz, \
             tc.tile_pool(name="pst", bufs=2, space="PSUM") as pst:
            nc.gpsimd.load_library(library_config.mlp)

            # per-superblock slices of everything load inside the sb loop so
            # the gather stream starts immediately and loads interleave
            idx_sb = sb.tile([IDXREP * 16, TOT // 16], I16)
            oix_sb = sb.tile([IDXREP * 16, j2 * P // 16], I16)
            nc.sync.dma_start(oix_sb[:], oidx[:])
            cl_sb = sb.tile([P, J], F32)
            wd_sb = sb.tile([P, J], BF)
            dv_sb = sb.tile([P, J], BF)
            wf_sb = sb.tile([P, J], F32)
            dt_sb = sb.tile([P, NB], F32)
            nc.sync.dma_start(dt_sb[:], dinv_t[:])
            w_sb = sb.tile([P, P], BF)
            nc.sync.dma_start(w_sb[:], wsb[:])
            nfl_sb = sb.tile([P, NSPAD], BF)
            ident_bf = sb.tile([P, P], BF)
            make_identity(nc, ident_bf[:])
            idxcol = []
            c0 = 0
            for sbi in range(NSB):
                ncols = 8 * sum(nch for s, q, nch, _, _ in call_specs
                                if s == sbi)
                idxcol.append((c0, c0 + ncols))
                c0 += ncols
            assert c0 == TOT // 16

            iota_i = sb.tile([P, P], mybir.dt.int32)
            nc.gpsimd.iota(iota_i[:], pattern=[[1, P]], channel_multiplier=0)
            iota_f = sb.tile([P, P], F32)
            nc.vector.tensor_copy(iota_f[:], iota_i[:])
            iota_bf = sb.tile([P, P], BF)
            nc.vector.tensor_copy(iota_bf[:], iota_f[:])

            road_sb = sb.tile([P, NSPAD], BF)
            og = sb.tile([P, (j2a + j2b) * P], BF)

            ci = 0  # call index
            for sbi in range(NSB):
                blks = SB_BLKS[sbi]
                i0, i1 = idxcol[sbi]
                if i1 > i0:
                    nc.sync.dma_start(idx_sb[:, i0:i1], idxs[:, i0:i1])
                gts = {}
                for q in range(NBANK):
                    if ci < len(call_specs) and call_specs[ci][0] == sbi \
                            and call_specs[ci][1] == q:
                        _, _, nch, slot0, _exact = call_specs[ci]
                        ci += 1
                        gt = gp.tile([P, nch * P], BF, tag=f"g{q}")
                        nc.gpsimd.dma_gather(
                            gt[:].rearrange("p (j d) -> p j d", d=P),
                            xview[:, q, :],
                            idx_sb[:, slot0 // 16:slot0 // 16 + nch * 8],
                            nch * P, nch * P, P, elem_step=NBANK * P,
                            single_packet=False)
                        gts[q] = (gt, slot0)
                if sbi == NSB - 1 and j2a:
                    # 96%% of trajectory rows reference blocks < SPLIT_B whose
                    # road slices are done; gather them under this superblock
                    nc.gpsimd.dma_gather(
                        og[:, :j2a * P].rearrange("p (c n) -> p c n", c=1),
                        road_sb[:, :SPLIT_B * P], oix_sb[:, :j2a * 8],
                        j2a * P, j2a * P, P,
                        transpose=True, single_packet=False,
                        sbuf_tokens_per_rank=P,
                        sbuf_free_dim_per_rank=P * 2,
                        sbuf_byte_offset=0)
                    nc.sync.dma_start(out_packed[:, :j2a * P],
                                      og[:, :j2a * P])
                cl0, cl1 = sbcols[sbi]
                if cl1 > cl0:
                    nc.sync.dma_start(cl_sb[:, cl0:cl1], clp[:, cl0:cl1])
                    nc.sync.dma_start(wd_sb[:, cl0:cl1], wdp[:, cl0:cl1])
                    nc.sync.dma_start(dv_sb[:, cl0:cl1], dvp[:, cl0:cl1])
                    nc.vector.tensor_tensor(
                        out=wf_sb[:, cl0:cl1], in0=wd_sb[:, cl0:cl1],
                        in1=dv_sb[:, cl0:cl1], op=mybir.AluOpType.mult)
                b_lo, b_hi = blks[0], blks[-1] + 1
                nc.sync.dma_start(nfl_sb[:, b_lo * P:b_hi * P],
                                  nfloc[:, b_lo * P:b_hi * P])
                # bank-major issue: PE never stalls on a later bank's gather
                # while earlier-bank work for other blocks is ready. Each
                # block's accumulator owns a full PSUM bank (multi-matmul
                # chains must not share a bank).
                blk0 = blks[0]
                zps = {b: psz.tile([P, P], F32, tag=f"zp{b - blk0}",
                                   name=f"zp{b - blk0}")
                       for b in blks}
                ji = {b: 0 for b in blks}
                tot = {b: int(novl[b].sum()) for b in blks}
                for q in range(NBANK):
                    for b in blks:
                        no = int(novl[b][q])
                        if no == 0:
                            continue
                        gt, slot0 = gts[q]
                        ch0 = (int(slotbase[b][q]) - slot0) // P
                        for lc in range(no):
                            col = int(colbase[b][q]) + lc
                            c = ch0 + lc
                            ohw = op_.tile([P, P], BF, tag="oh")
                            nc.vector.tensor_scalar(
                                ohw[:], iota_bf[:], cl_sb[:, col:col + 1],
                                wf_sb[:, col:col + 1],
                                mybir.AluOpType.is_equal, mybir.AluOpType.mult)
                            # z[d, c] += sum_p gt[p, d] * ohw[p, c]   (s^T)
                            nc.tensor.matmul(
                                zps[b][:], lhsT=gt[:, c * P:(c + 1) * P],
                                rhs=ohw[:],
                                start=(ji[b] == 0), stop=False)
                            ji[b] += 1
                # self-loop term: z[d, c] += dinv[c] * nf_local[c, d]
                for b in blks:
                    ohd = op_.tile([P, P], BF, tag="oh")
                    nc.vector.tensor_scalar(
                        ohd[:], ident_bf[:], dt_sb[:, b:b + 1], None,
                        mybir.AluOpType.mult)
                    nc.tensor.matmul(
                        zps[b][:], lhsT=nfl_sb[:, b * P:(b + 1) * P],
                        rhs=ohd[:], start=(ji[b] == 0), stop=True)
                quads = [list(blks)[i:i + 4]
                         for i in range(0, len(blks), 4)]
                for quad in quads:
                    tpq = pst.tile([P, 4 * P], F32, tag="tq", name="tpq")
                    for j, b in enumerate(quad):
                        sT = op_.tile([P, P], BF, tag="sT")
                        nc.scalar.activation(
                            sT[:], zps[b][:],
                            mybir.ActivationFunctionType.Copy)
                        nc.tensor.matmul(tpq[:, j * P:(j + 1) * P],
                                         lhsT=sT[:], rhs=w_sb[:],
                                         start=True, stop=True)
                    for j, b in enumerate(quad):
                        nc.scalar.activation(
                            road_sb[:, b * P:(b + 1) * P],
                            tpq[:, j * P:(j + 1) * P],
                            mybir.ActivationFunctionType.Relu,
                            scale=dt_sb[:, b:b + 1])

            # remaining trajectory rows (blocks >= SPLIT_B, idx rebased)
            if j2b:
                nc.gpsimd.dma_gather(
                    og[:, j2a * P:].rearrange("p (c n) -> p c n", c=1),
                    road_sb[:, SPLIT_B * P:], oix_sb[:, j2a * 8:],
                    j2b * P, j2b * P, P,
                    transpose=True, single_packet=False,
                    sbuf_tokens_per_rank=P,
                    sbuf_free_dim_per_rank=P * 2,
                    sbuf_byte_offset=0)
                nc.sync.dma_start(out_packed[:, j2a * P:], og[:, j2a * P:])
    nc.compile()
    return nc


def kernel(**inputs):
    traj = np.asarray(inputs["traj_seqs"])[..., 0].astype(np.int64)
    seq_len = np.asarray(inputs["seq_len"]).astype(np.int64)
    nf = np.asarray(inputs["node_feat"], dtype=np.float32)
    ei = np.asarray(inputs["edge_index"]).astype(np.int64)
    ef = np.asarray(inputs["edge_feat"], dtype=np.float32)
    W = np.asarray(inputs["W"], dtype=np.float32)
    b = np.asarray(inputs["b"], dtype=np.float32)
    assert np.all(b == 0.0), "nonzero bias not wired into device path"

    row, col = ei[0], ei[1]
    owner = col // NS

    # ---------- per-core edge sets (self-loops handled densely on-chip) ----
    core_edges = []
    cnts = np.zeros((NCORES, NB, NBANK), np.int64)
    for k in range(NCORES):
        m = owner == k
        ck = col[m] - k * NS
        rk = row[m]
        wk = ef[m]
        bq = (ck // P) * NBANK + (rk % NBANK)
        srt = np.argsort(bq, kind="stable")
        ck, rk, wk, bq = ck[srt], rk[srt], wk[srt], bq[srt]
        np.add.at(cnts, (k, bq // NBANK, bq % NBANK), 1)
        core_edges.append((ck, rk, wk, bq))

    cap = cnts.max(axis=0)  # [NB, NBANK] tight group capacities
    _, colbase, novl, slotbase, J, TOT, _sbcols = _schedule(cap)

    # trajectory selection
    flat = traj.reshape(-1)
    L = traj.shape[1]
    posmask = (np.arange(L)[None, :] < seq_len[:, None]).reshape(-1)
    oo = flat // NS
    SPLIT_B = NB - SB_SIZES[-1]
    sels_a, sels_b = [], []
    for k in range(NCORES):
        s = np.where((oo == k) & posmask)[0]
        lv = flat[s] - k * NS
        sels_a.append(s[lv < SPLIT_B * P])
        sels_b.append(s[lv >= SPLIT_B * P])
    j2a = max(1, int(np.ceil(max(len(s) for s in sels_a) / P)))
    j2b = max(1, int(np.ceil(max(len(s) for s in sels_b) / P)))

    # ---------- launch A (deg/dinv only) ----------
    padw = 1
    for k in range(NCORES):
        m = owner == k
        c_loc = col[m] - k * NS
        padw = max(padw, int(np.bincount(c_loc, minlength=NS).max()))

    x_full = np.ascontiguousarray(nf.astype(BF16))   # gather source (raw feats)
    W_bf = np.ascontiguousarray(W.astype(BF16))
    in_maps_a = []
    for k in range(NCORES):
        m = owner == k
        c_loc = col[m] - k * NS
        w_loc = ef[m]
        cnt = np.bincount(c_loc, minlength=NS)
        starts = np.zeros(NS, np.int64)
        np.cumsum(cnt[:-1], out=starts[1:])
        srt = np.argsort(c_loc, kind="stable")
        cs, ws = c_loc[srt], w_loc[srt]
        posin = np.arange(len(cs)) - starts[cs]
        arr = np.zeros((NSPAD, padw), BF16)
        arr[cs, posin] = ws.astype(BF16)
        wpad = np.ascontiguousarray(
            arr.reshape(NB, P, padw).transpose(1, 0, 2).reshape(P, NB * padw))
        in_maps_a.append({"wpad": wpad})

    nca = _build_kernel_a(padw)
    ra = run_bass_kernel_spmd(nca, in_maps_a, core_ids=list(range(NCORES)))

    dinv_full = np.zeros(N, np.float32)
    for k in range(NCORES):
        ds = ra.results[k]["dinv_sh"]       # [128, NB]
        dr = ds.T.reshape(NSPAD)
        dinv_full[k * NS:(k + 1) * NS] = dr[:NS]

    # ---------- launch B ----------
    in_maps_b = []
    for k in range(NCORES):
        ck, rk, wk, bq = core_edges[k]
        bqcnt = np.bincount(bq, minlength=NB * NBANK).reshape(NB, NBANK)
        gstart = np.zeros(NB * NBANK, np.int64)
        np.cumsum(bqcnt.reshape(-1)[:-1], out=gstart[1:])
        pos = np.arange(len(ck)) - gstart[bq]
        sbase = slotbase[bq // NBANK, bq % NBANK]
        f = sbase + pos                       # global flat slot
        col = colbase[bq // NBANK, bq % NBANK] + (f // P - sbase // P)
        par = f % P

        clp = np.zeros((P, J), np.float32)
        wdp = np.zeros((P, J), BF16)
        dvp = np.zeros((P, J), BF16)
        clp[par, col] = (ck % P).astype(np.float32)
        wdp[par, col] = wk.astype(BF16)
        dvp[par, col] = dinv_full[rk].astype(BF16)

        idx_arr = np.zeros((16, TOT // 16), np.int16)
        idx_arr[f % 16, f // 16] = (rk // NBANK).astype(np.int16)
        idx_t = np.tile(idx_arr, (IDXREP, 1))

        lva = (flat[sels_a[k]] - k * NS).astype(np.int16)
        lvb = (flat[sels_b[k]] - k * NS - SPLIT_B * P).astype(np.int16)
        oarr = np.zeros((16, (j2a + j2b) * P // 16), np.int16)
        fa = np.arange(len(lva))
        oarr[fa % 16, fa // 16] = lva
        fb = np.arange(len(lvb))
        oarr[fb % 16, j2a * 8 + fb // 16] = lvb
        oidx_t = np.tile(oarr, (IDXREP, 1))

        nfl = np.zeros((NSPAD, P), BF16)
        nfl[:NS] = x_full[k * NS:(k + 1) * NS]
        nfl = np.ascontiguousarray(
            nfl.reshape(NB, P, P).transpose(1, 0, 2).reshape(P, NSPAD))
        in_maps_b.append({
            "xfull": x_full, "wsb": W_bf, "idxs": idx_t, "oidx": oidx_t,
            "clp": clp, "wdp": wdp, "dvp": dvp,
            "dinv_t": ra.results[k]["dinv_sh"], "nfloc": nfl,
        })

    ncb = _build_kernel_b(cap, j2a, j2b)
    rb = run_bass_kernel_spmd(ncb, in_maps_b, core_ids=list(range(NCORES)))

    global LAST_EXEC_NS, LAST_EXEC_PARTS, LAST_NCS
    LAST_NCS = (nca, ncb)
    LAST_EXEC_PARTS = (ra.exec_time_ns, rb.exec_time_ns)
    if ra.exec_time_ns and rb.exec_time_ns:
        LAST_EXEC_NS = ra.exec_time_ns + rb.exec_time_ns

    out = np.zeros((flat.shape[0], D), np.float32)
    for k in range(NCORES):
        op = rb.results[k]["out_packed"]
        if len(sels_a[k]):
            out[sels_a[k]] = op[:, :len(sels_a[k])].T.astype(np.float32)
        if len(sels_b[k]):
            out[sels_b[k]] = op[:, j2a * P:j2a * P + len(sels_b[k])].T.astype(
                np.float32)
    return out.reshape(traj.shape[0], L, D)


# revision 50
# speedup vs baseline: 1.2558x; 1.0091x over previous
"""Trainium2 Bass kernel for nn_LocationEmbedding (GCN scatter-add + trajectory gather).

Strategy (8 NeuronCores, SPMD, two launches):
  Launch A (per core, owns nodes [k*12500, (k+1)*12500)):
    deg via segmented reduce of host-padded bf16 edge weights,
    dinv = rsqrt(deg + 1). Host assembles dinv_full [100000] f32.
  Launch B (per core, target-sharded edges; self-loops handled densely):
    - edges grouped by (target 128-block, source bank = row % 4; 4 banks so
      gather idxs fit int16), tight-packed into per-(superblock, bank)
      dma_gather calls over strided bank views of bf16 node features; a
      128-slot chunk may straddle two blocks (one matmul per block, zero
      weights masking foreign slots)
    - per chunk column: one fused DVE op builds the weighted one-hot
      (iota == cl) * w', w' = w * dinv[row] (computed on device); matmul
      (lhsT=gathered, rhs=one-hot) accumulates s^T per block in PSUM,
      bank-major so the PE never stalls on a later bank's gather; each
      block's accumulator owns a full PSUM bank
    - self-loop term added per block via a dense diagonal matmul from a
      locally-loaded feature tile (no gather slots spent on self-loops)
    - block tail on the Activation engine: sT = copy(z^T), t = sT @ W,
      road = Relu(t, scale=dinv_t)
    - trajectory rows fetched by an SBUF-source transpose dma_gather from
      the road tile; output returned transposed, host transposes back
    All per-superblock loads (idxs, cl/w/dinv metadata, local features)
    interleave with the gather stream instead of serializing at the head.
All arithmetic on device; host does sharding, padding, and index layout.
"""

import numpy as np
import ml_dtypes

import concourse.bass as bass
import concourse.bacc as bacc
import concourse.tile as tile
from concourse import mybir, library_config
from concourse.bass_utils import run_bass_kernel_spmd
from concourse.masks import make_identity

BF16 = ml_dtypes.bfloat16
P = 128
N, E, D = 100000, 1600000, 128
NCORES = 8
NS = N // NCORES          # 12500 nodes per core
NB = (NS + P - 1) // P    # 98 target blocks per core
NSPAD = NB * P            # 12544
NBANK = 4                 # source banks (row % 4) so gather idxs fit int16
# superblock sizes (blocks per gather round); small first/last shrink the
# pipeline head/tail; max 6 so each block's PSUM accumulator owns a bank
SB_SIZES = [4] + [6] * 15 + [4]
assert sum(SB_SIZES) == NB
SB_BLKS = []
_b0 = 0
for _s in SB_SIZES:
    SB_BLKS.append(range(_b0, _b0 + _s))
    _b0 += _s
NSB = len(SB_BLKS)
IDXREP = 2                # idx tiles replicated to 2x16 partitions

F32 = mybir.dt.float32
BF = mybir.dt.bfloat16
I16 = mybir.dt.int16

LAST_EXEC_NS = None
LAST_EXEC_PARTS = None
LAST_NCS = None  # (nca, ncb) for offline simulation


def _build_kernel_a(padw):
    """deg/dinv only: dinv = rsqrt(1 + segmented-sum of edge weights)."""
    nc = bacc.Bacc("TRN2", target_bir_lowering=False, debug=False)
    wpad = nc.dram_tensor("wpad", [P, NB * padw], BF, kind="ExternalInput")
    dinv_sh = nc.dram_tensor("dinv_sh", [P, NB], F32, kind="ExternalOutput")
    with tile.TileContext(nc) as tc:
        with tc.tile_pool(name="sb", bufs=1) as sb:
            wp_sb = sb.tile([P, NB * padw], BF)
            deg = sb.tile([P, NB], F32)
            half = (NB // 2) * padw
            for lo, hi, b0, b1 in ((0, half, 0, NB // 2),
                                   (half, NB * padw, NB // 2, NB)):
                nc.sync.dma_start(wp_sb[:, lo:hi], wpad[:, lo:hi])
                nc.vector.tensor_reduce(
                    out=deg[:, b0:b1],
                    in_=wp_sb[:, lo:hi].rearrange("p (b s) -> p b s", s=padw),
                    axis=mybir.AxisListType.X,
                    op=mybir.AluOpType.add,
                )
            nc.vector.tensor_scalar_add(deg[:], deg[:], 1.0)
            rec = sb.tile([P, NB], F32)
            nc.vector.reciprocal(rec[:], deg[:])
            dinv = sb.tile([P, NB], F32)
            nc.scalar.activation(dinv[:], rec[:], mybir.ActivationFunctionType.Sqrt)
            nc.sync.dma_start(dinv_sh[:], dinv[:])
    nc.compile()
    return nc


def _schedule(cap):
    """Tight-packed gather schedule, identical across cores.

    cap: [NB][NBANK] slot capacity per (block, bank) group (max over cores).
    Groups pack back-to-back inside each (superblock, bank) call; chunks are
    fixed 128-slot slices of the call, so a chunk can span two adjacent
    blocks (it then feeds one matmul per block, with zero weights masking
    the other block's slots).
    Returns (calls, colbase, novl, slotbase, J2, TOT):
      calls: (sbi, q, nch, slot0) with slot0 the call's global slot base
      colbase/novl: per (b,q) first metadata column and #overlapped chunks
      slotbase: per (b,q) global slot of the group start
      J2: total metadata columns; TOT: total padded slots
    """
    cap = np.asarray(cap)
    calls = []  # (sbi, q, nch, slot0, exact_slots)
    colbase = np.zeros((NB, NBANK), np.int64)
    novl = np.zeros((NB, NBANK), np.int64)
    slotbase = np.zeros((NB, NBANK), np.int64)
    sbcols = []
    col = 0
    slot0 = 0
    for sbi in range(NSB):
        blks = SB_BLKS[sbi]
        col_lo = col
        for q in range(NBANK):
            caps = [(b, int(cap[b][q])) for b in blks]
            total = sum(c for _, c in caps)
            if total == 0:
                continue
            nch = (total + P - 1) // P
            exact = total
            S = 0
            for b, c in caps:
                slotbase[b][q] = slot0 + S
                if c > 0:
                    colbase[b][q] = col
                    novl[b][q] = (S + c - 1) // P - S // P + 1
                    col += int(novl[b][q])
                S += c
            calls.append((sbi, q, nch, slot0, exact))
            slot0 += nch * P
        sbcols.append((col_lo, col))
    return calls, colbase, novl, slotbase, int(col), int(slot0), sbcols


SPLIT_B = NB - SB_SIZES[-1]   # trajectory rows below this block gather early


def _build_kernel_b(cap, j2a, j2b):
    """cap: [NB][NBANK] group capacities (identical across cores); j2a/j2b:
    output gather chunks for road rows below/above SPLIT_B*128."""
    j2 = j2a + j2b
    call_specs, colbase, novl, slotbase, J, TOT, sbcols = _schedule(cap)

    nc = bacc.Bacc("TRN2", target_bir_lowering=False, debug=False)
    xfull = nc.dram_tensor("xfull", [N, P], BF, kind="ExternalInput")
    wsb = nc.dram_tensor("wsb", [P, P], BF, kind="ExternalInput")
    idxs = nc.dram_tensor("idxs", [IDXREP * 16, TOT // 16], I16,
                          kind="ExternalInput")
    oidx = nc.dram_tensor("oidx", [IDXREP * 16, j2 * P // 16], I16,
                          kind="ExternalInput")
    clp = nc.dram_tensor("clp", [P, J], BF, kind="ExternalInput")
    wdp = nc.dram_tensor("wdp", [P, J], BF, kind="ExternalInput")
    dvp = nc.dram_tensor("dvp", [P, J], BF, kind="ExternalInput")
    dinv_t = nc.dram_tensor("dinv_t", [P, NB], F32, kind="ExternalInput")
    nfloc = nc.dram_tensor("nfloc", [P, NSPAD], BF, kind="ExternalInput")
    out_packed = nc.dram_tensor("out_packed", [P, j2 * P], BF,
                                kind="ExternalOutput")

    # bank view of xfull: rows r with r % NBANK == q, idx = r // NBANK
    xview = xfull[:].rearrange("(n f) d -> n f d", f=NBANK)

    with tile.TileContext(nc) as tc:
        with tc.tile_pool(name="sb", bufs=1) as sb, \
             tc.tile_pool(name="gp", bufs=2) as gp, \
             tc.tile_pool(name="op", bufs=20) as op_, \
             tc.tile_pool(name="psz", bufs=1, space="PSUM") as psz, \
             tc.tile_pool(name="pst", bufs=2, space="PSUM") as pst:
            nc.gpsimd.load_library(library_config.mlp)

            # per-superblock slices of everything load inside the sb loop so
            # the gather stream starts immediately and loads interleave
            idx_sb = sb.tile([IDXREP * 16, TOT // 16], I16)
            oix_sb = sb.tile([IDXREP * 16, j2 * P // 16], I16)
            nc.sync.dma_start(oix_sb[:], oidx[:])
            cl_sb = sb.tile([P, J], F32)
            clb_sb = sb.tile([P, J], BF)
            wd_sb = sb.tile([P, J], BF)
            dv_sb = sb.tile([P, J], BF)
            wf_sb = sb.tile([P, J], F32)
            dt_sb = sb.tile([P, NB], F32)
            nc.sync.dma_start(dt_sb[:], dinv_t[:])
            w_sb = sb.tile([P, P], BF)
            nc.sync.dma_start(w_sb[:], wsb[:])
            nfl_sb = sb.tile([P, NSPAD], BF)
            ident_bf = sb.tile([P, P], BF)
            make_identity(nc, ident_bf[:])
            idxcol = []
            c0 = 0
            for sbi in range(NSB):
                ncols = 8 * sum(nch for s, q, nch, _, _ in call_specs
                                if s == sbi)
                idxcol.append((c0, c0 + ncols))
                c0 += ncols
            assert c0 == TOT // 16

            iota_i = sb.tile([P, P], mybir.dt.int32)
            nc.gpsimd.iota(iota_i[:], pattern=[[1, P]], channel_multiplier=0)
            iota_f = sb.tile([P, P], F32)
            nc.vector.tensor_copy(iota_f[:], iota_i[:])
            iota_bf = sb.tile([P, P], BF)
            nc.vector.tensor_copy(iota_bf[:], iota_f[:])

            road_sb = sb.tile([P, NSPAD], BF)
            og = sb.tile([P, (j2a + j2b) * P], BF)

            ci = 0  # call index
            for sbi in range(NSB):
                blks = SB_BLKS[sbi]
                i0, i1 = idxcol[sbi]
                if i1 > i0:
                    nc.sync.dma_start(idx_sb[:, i0:i1], idxs[:, i0:i1])
                gts = {}
                for q in range(NBANK):
                    if ci < len(call_specs) and call_specs[ci][0] == sbi \
                            and call_specs[ci][1] == q:
                        _, _, nch, slot0, _exact = call_specs[ci]
                        ci += 1
                        gt = gp.tile([P, nch * P], BF, tag=f"g{q}")
                        nc.gpsimd.dma_gather(
                            gt[:].rearrange("p (j d) -> p j d", d=P),
                            xview[:, q, :],
                            idx_sb[:, slot0 // 16:slot0 // 16 + nch * 8],
                            nch * P, nch * P, P, elem_step=NBANK * P,
                            single_packet=False)
                        gts[q] = (gt, slot0)
                if sbi == NSB - 1 and j2a:
                    # 96%% of trajectory rows reference blocks < SPLIT_B whose
                    # road slices are done; gather them under this superblock
                    nc.gpsimd.dma_gather(
                        og[:, :j2a * P].rearrange("p (c n) -> p c n", c=1),
                        road_sb[:, :SPLIT_B * P], oix_sb[:, :j2a * 8],
                        j2a * P, j2a * P, P,
                        transpose=True, single_packet=False,
                        sbuf_tokens_per_rank=P,
                        sbuf_free_dim_per_rank=P * 2,
                        sbuf_byte_offset=0)
                    nc.sync.dma_start(out_packed[:, :j2a * P],
                                      og[:, :j2a * P])
                cl0, cl1 = sbcols[sbi]
                if cl1 > cl0:
                    nc.sync.dma_start(clb_sb[:, cl0:cl1], clp[:, cl0:cl1])
                    nc.vector.tensor_copy(cl_sb[:, cl0:cl1],
                                          clb_sb[:, cl0:cl1])
                    nc.sync.dma_start(wd_sb[:, cl0:cl1], wdp[:, cl0:cl1])
                    nc.sync.dma_start(dv_sb[:, cl0:cl1], dvp[:, cl0:cl1])
                    nc.vector.tensor_tensor(
                        out=wf_sb[:, cl0:cl1], in0=wd_sb[:, cl0:cl1],
                        in1=dv_sb[:, cl0:cl1], op=mybir.AluOpType.mult)
                b_lo, b_hi = blks[0], blks[-1] + 1
                nc.sync.dma_start(nfl_sb[:, b_lo * P:b_hi * P],
                                  nfloc[:, b_lo * P:b_hi * P])
                # bank-major issue: PE never stalls on a later bank's gather
                # while earlier-bank work for other blocks is ready. Each
                # block's accumulator owns a full PSUM bank (multi-matmul
                # chains must not share a bank).
                blk0 = blks[0]
                zps = {b: psz.tile([P, P], F32, tag=f"zp{b - blk0}",
                                   name=f"zp{b - blk0}")
                       for b in blks}
                ji = {b: 0 for b in blks}
                tot = {b: int(novl[b].sum()) for b in blks}
                for q in range(NBANK):
                    for b in blks:
                        no = int(novl[b][q])
                        if no == 0:
                            continue
                        gt, slot0 = gts[q]
                        ch0 = (int(slotbase[b][q]) - slot0) // P
                        for lc in range(no):
                            col = int(colbase[b][q]) + lc
                            c = ch0 + lc
                            ohw = op_.tile([P, P], BF, tag="oh")
                            nc.vector.tensor_scalar(
                                ohw[:], iota_bf[:], cl_sb[:, col:col + 1],
                                wf_sb[:, col:col + 1],
                                mybir.AluOpType.is_equal, mybir.AluOpType.mult)
                            # z[d, c] += sum_p gt[p, d] * ohw[p, c]   (s^T)
                            nc.tensor.matmul(
                                zps[b][:], lhsT=gt[:, c * P:(c + 1) * P],
                                rhs=ohw[:],
                                start=(ji[b] == 0), stop=False)
                            ji[b] += 1
                # self-loop term: z[d, c] += dinv[c] * nf_local[c, d]
                for b in blks:
                    ohd = op_.tile([P, P], BF, tag="oh")
                    nc.vector.tensor_scalar(
                        ohd[:], ident_bf[:], dt_sb[:, b:b + 1], None,
                        mybir.AluOpType.mult)
                    nc.tensor.matmul(
                        zps[b][:], lhsT=nfl_sb[:, b * P:(b + 1) * P],
                        rhs=ohd[:], start=(ji[b] == 0), stop=True)
                quads = [list(blks)[i:i + 4]
                         for i in range(0, len(blks), 4)]
                for quad in quads:
                    tpq = pst.tile([P, 4 * P], F32, tag="tq", name="tpq")
                    for j, b in enumerate(quad):
                        sT = op_.tile([P, P], BF, tag="sT")
                        nc.scalar.activation(
                            sT[:], zps[b][:],
                            mybir.ActivationFunctionType.Copy)
                        nc.tensor.matmul(tpq[:, j * P:(j + 1) * P],
                                         lhsT=sT[:], rhs=w_sb[:],
                                         start=True, stop=True)
                    for j, b in enumerate(quad):
                        nc.scalar.activation(
                            road_sb[:, b * P:(b + 1) * P],
                            tpq[:, j * P:(j + 1) * P],
                            mybir.ActivationFunctionType.Relu,
                            scale=dt_sb[:, b:b + 1])

            # remaining trajectory rows (blocks >= SPLIT_B, idx rebased)
            if j2b:
                nc.gpsimd.dma_gather(
                    og[:, j2a * P:].rearrange("p (c n) -> p c n", c=1),
                    road_sb[:, SPLIT_B * P:], oix_sb[:, j2a * 8:],
                    j2b * P, j2b * P, P,
                    transpose=True, single_packet=False,
                    sbuf_tokens_per_rank=P,
                    sbuf_free_dim_per_rank=P * 2,
                    sbuf_byte_offset=0)
                nc.sync.dma_start(out_packed[:, j2a * P:], og[:, j2a * P:])
    nc.compile()
    return nc


def kernel(**inputs):
    traj = np.asarray(inputs["traj_seqs"])[..., 0].astype(np.int64)
    seq_len = np.asarray(inputs["seq_len"]).astype(np.int64)
    nf = np.asarray(inputs["node_feat"], dtype=np.float32)
    ei = np.asarray(inputs["edge_index"]).astype(np.int64)
    ef = np.asarray(inputs["edge_feat"], dtype=np.float32)
    W = np.asarray(inputs["W"], dtype=np.float32)
    b = np.asarray(inputs["b"], dtype=np.float32)
    assert np.all(b == 0.0), "nonzero bias not wired into device path"

    row, col = ei[0], ei[1]
    owner = col // NS

    # ---------- per-core edge sets (self-loops handled densely on-chip) ----
    core_edges = []
    cnts = np.zeros((NCORES, NB, NBANK), np.int64)
    for k in range(NCORES):
        m = owner == k
        ck = col[m] - k * NS
        rk = row[m]
        wk = ef[m]
        bq = (ck // P) * NBANK + (rk % NBANK)
        srt = np.argsort(bq, kind="stable")
        ck, rk, wk, bq = ck[srt], rk[srt], wk[srt], bq[srt]
        np.add.at(cnts, (k, bq // NBANK, bq % NBANK), 1)
        core_edges.append((ck, rk, wk, bq))

    cap = cnts.max(axis=0)  # [NB, NBANK] tight group capacities
    _, colbase, novl, slotbase, J, TOT, _sbcols = _schedule(cap)

    # trajectory selection
    flat = traj.reshape(-1)
    L = traj.shape[1]
    posmask = (np.arange(L)[None, :] < seq_len[:, None]).reshape(-1)
    oo = flat // NS
    SPLIT_B = NB - SB_SIZES[-1]
    sels_a, sels_b = [], []
    for k in range(NCORES):
        s = np.where((oo == k) & posmask)[0]
        lv = flat[s] - k * NS
        sels_a.append(s[lv < SPLIT_B * P])
        sels_b.append(s[lv >= SPLIT_B * P])
    j2a = max(1, int(np.ceil(max(len(s) for s in sels_a) / P)))
    j2b = max(1, int(np.ceil(max(len(s) for s in sels_b) / P)))

    # ---------- launch A (deg/dinv only) ----------
    padw = 1
    for k in range(NCORES):
        m = owner == k
        c_loc = col[m] - k * NS
        padw = max(padw, int(np.bincount(c_loc, minlength=NS).max()))

    x_full = np.ascontiguousarray(nf.astype(BF16))   # gather source (raw feats)
    W_bf = np.ascontiguousarray(W.astype(BF16))
    in_maps_a = []
    for k in range(NCORES):
        m = owner == k
        c_loc = col[m] - k * NS
        w_loc = ef[m]
        cnt = np.bincount(c_loc, minlength=NS)
        starts = np.zeros(NS, np.int64)
        np.cumsum(cnt[:-1], out=starts[1:])
        srt = np.argsort(c_loc, kind="stable")
        cs, ws = c_loc[srt], w_loc[srt]
        posin = np.arange(len(cs)) - starts[cs]
        arr = np.zeros((NSPAD, padw), BF16)
        arr[cs, posin] = ws.astype(BF16)
        wpad = np.ascontiguousarray(
            arr.reshape(NB, P, padw).transpose(1, 0, 2).reshape(P, NB * padw))
        in_maps_a.append({"wpad": wpad})

    nca = _build_kernel_a(padw)
    ra = run_bass_kernel_spmd(nca, in_maps_a, core_ids=list(range(NCORES)))

    dinv_full = np.zeros(N, np.float32)
    for k in range(NCORES):
        ds = ra.results[k]["dinv_sh"]       # [128, NB]
        dr = ds.T.reshape(NSPAD)
        dinv_full[k * NS:(k + 1) * NS] = dr[:NS]

    # ---------- launch B ----------
    in_maps_b = []
    for k in range(NCORES):
        ck, rk, wk, bq = core_edges[k]
        bqcnt = np.bincount(bq, minlength=NB * NBANK).reshape(NB, NBANK)
        gstart = np.zeros(NB * NBANK, np.int64)
        np.cumsum(bqcnt.reshape(-1)[:-1], out=gstart[1:])
        pos = np.arange(len(ck)) - gstart[bq]
        sbase = slotbase[bq // NBANK, bq % NBANK]
        f = sbase + pos                       # global flat slot
        col = colbase[bq // NBANK, bq % NBANK] + (f // P - sbase // P)
        par = f % P

        clp = np.zeros((P, J), BF16)
        wdp = np.zeros((P, J), BF16)
        dvp = np.zeros((P, J), BF16)
        clp[par, col] = (ck % P).astype(BF16)
        wdp[par, col] = wk.astype(BF16)
        dvp[par, col] = dinv_full[rk].astype(BF16)

        idx_arr = np.zeros((16, TOT // 16), np.int16)
        idx_arr[f % 16, f // 16] = (rk // NBANK).astype(np.int16)
        idx_t = np.tile(idx_arr, (IDXREP, 1))

        lva = (flat[sels_a[k]] - k * NS).astype(np.int16)
        lvb = (flat[sels_b[k]] - k * NS - SPLIT_B * P).astype(np.int16)
        oarr = np.zeros((16, (j2a + j2b) * P // 16), np.int16)
        fa = np.arange(len(lva))
        oarr[fa % 16, fa // 16] = lva
        fb = np.arange(len(lvb))
        oarr[fb % 16, j2a * 8 + fb // 16] = lvb
        oidx_t = np.tile(oarr, (IDXREP, 1))

        nfl = np.zeros((NSPAD, P), BF16)
        nfl[:NS] = x_full[k * NS:(k + 1) * NS]
        nfl = np.ascontiguousarray(
            nfl.reshape(NB, P, P).transpose(1, 0, 2).reshape(P, NSPAD))
        in_maps_b.append({
            "xfull": x_full, "wsb": W_bf, "idxs": idx_t, "oidx": oidx_t,
            "clp": clp, "wdp": wdp, "dvp": dvp,
            "dinv_t": ra.results[k]["dinv_sh"], "nfloc": nfl,
        })

    ncb = _build_kernel_b(cap, j2a, j2b)
    rb = run_bass_kernel_spmd(ncb, in_maps_b, core_ids=list(range(NCORES)))

    global LAST_EXEC_NS, LAST_EXEC_PARTS, LAST_NCS
    LAST_NCS = (nca, ncb)
    LAST_EXEC_PARTS = (ra.exec_time_ns, rb.exec_time_ns)
    if ra.exec_time_ns and rb.exec_time_ns:
        LAST_EXEC_NS = ra.exec_time_ns + rb.exec_time_ns

    out = np.zeros((flat.shape[0], D), np.float32)
    for k in range(NCORES):
        op = rb.results[k]["out_packed"]
        if len(sels_a[k]):
            out[sels_a[k]] = op[:, :len(sels_a[k])].T.astype(np.float32)
        if len(sels_b[k]):
            out[sels_b[k]] = op[:, j2a * P:j2a * P + len(sels_b[k])].T.astype(
                np.float32)
    return out.reshape(traj.shape[0], L, D)
